# revision 1
# baseline (speedup 1.0000x reference)
"""GAT (2-head, 64-ch) + BatchNorm message-passing kernel on 8 Trainium2 cores.

Dst-node graph-parallel sharding: 12500 dst nodes/core (padded 12544 = 98
blocks x 128). Phase 0 computes h = x @ W per shard (fp16) and AllGathers a
global gather table. Edges are routed to the dst core, grouped by dst block,
split into 4 source-quartile streams (dma_gather int16 index limit) padded to
128-edge chunks; self-loops form one sequential chunk per block. Per chunk:
dma_gather h[src] rows; a_src via PE-transposed matmul with att vectors;
a_dst expanded from the block's self-loop scores via a transposed eq-mask
matmul; exp(leaky_relu(.)) on ACT; per-head exp-weighted one-hot masks on
DVE; PE matmuls accumulate the softmax numerator and denominator in PSUM.
Per block: out = relu(agg/s); BN stats via ones-matmuls; stats AllReduced;
affine applied; result written out.
"""
import sys
sys.path.insert(0, "/opt/trn_rl_repo")
import numpy as np

N = 100_000
F = 128
H = 2
C = 64
HC = H * C
NEG_SLOPE = 0.2
BN_EPS = 1e-5
NCORES = 8
NSH_RAW = 12_500
NSH = 12_544          # 98 * 128
NB = NSH // 128       # 98
NTAB = NCORES * NSH   # 100352
QS = 4
QROWS = NTAB // QS    # 25088 (= 2 cores' shards; quartile = src_core // 2)
P = 128
BATCH_CHUNKS = 16     # chunks per dma_gather (2048 rows)
PADVAL = 200.0


def _g_chunk_base(i, qq, K, start_q):
    off = 0
    for q2 in range(qq):
        off += int(K[:, q2].sum())
    return off + int(start_q[i, qq])


def _host_prep(x, edge_index, W, att_src, att_dst, bias, gamma, beta):
    src = np.asarray(edge_index[0]).astype(np.int64)
    dst = np.asarray(edge_index[1]).astype(np.int64)
    x = np.asarray(x, dtype=np.float32)

    core_of = dst // NSH_RAW
    q_of = (src // NSH_RAW) // 2

    Kraw = np.zeros((NCORES, NB, QS), dtype=np.int64)
    core_edges = []
    for c in range(NCORES):
        m = core_of == c
        s_c = src[m]
        d_c = dst[m] - c * NSH_RAW
        q_c = q_of[m]
        rb_c = d_c // 128
        core_edges.append((s_c, d_c, q_c, rb_c))
        for qq in range(QS):
            cnt = np.bincount(rb_c[q_c == qq], minlength=NB)
            Kraw[c, :, qq] = (cnt + 127) // 128

    perm_blocks = np.zeros((NCORES, NB), dtype=np.int64)
    for c in range(NCORES):
        tot = Kraw[c].sum(axis=1)
        perm_blocks[c] = np.argsort(-tot, kind="stable")
    inv_perm = np.zeros((NCORES, NB), dtype=np.int64)
    for c in range(NCORES):
        inv_perm[c, perm_blocks[c]] = np.arange(NB)

    # uniform chunk counts per sorted block index
    K = np.zeros((NB, QS), dtype=np.int64)
    for qq in range(QS):
        per_core_sorted = np.stack(
            [Kraw[c, perm_blocks[c], qq] for c in range(NCORES)], axis=0)
        K[:, qq] = per_core_sorted.max(axis=0)
    Sq = K.sum(axis=0)
    start_q = np.zeros((NB, QS), dtype=np.int64)
    for qq in range(QS):
        start_q[1:, qq] = np.cumsum(K[:-1, qq])
    nbatch_q = [(int(Sq[qq]) + BATCH_CHUNKS - 1) // BATCH_CHUNKS
                for qq in range(QS)]
    TOTCH = int(Sq.sum())

    # global-table row of a global node id (after per-core block permutation)
    def table_row(g):
        c_s = g // NSH_RAW
        loc = g - c_s * NSH_RAW
        rb = loc // 128
        return c_s * NSH + inv_perm[c_s, rb] * 128 + (loc - rb * 128)

    attA = np.zeros((F, 4), dtype=np.float32)
    attA[0:C, 0] = np.asarray(att_src, dtype=np.float32)[0]
    attA[C:HC, 1] = np.asarray(att_src, dtype=np.float32)[1]
    attA[0:C, 2] = np.asarray(att_dst, dtype=np.float32)[0]
    attA[C:HC, 3] = np.asarray(att_dst, dtype=np.float32)[1]
    gbb = np.zeros((1, 3 * HC), dtype=np.float32)
    gbb[0, 0:HC] = np.asarray(gamma, dtype=np.float32).reshape(-1)
    gbb[0, HC:2 * HC] = np.asarray(beta, dtype=np.float32).reshape(-1)
    gbb[0, 2 * HC:] = np.asarray(bias, dtype=np.float32).reshape(-1)

    per_core = []
    for c in range(NCORES):
        s_c, d_c, q_c, rb_c = core_edges[c]
        i_c = inv_perm[c, rb_c]
        dl_c = (d_c - rb_c * 128).astype(np.int64)
        tr_c = table_row(s_c)
        order = np.lexsort((tr_c, q_c, i_c))
        q_o, i_o, dl_o, tr_o = q_c[order], i_c[order], dl_c[order], tr_c[order]

        idx16_q = [np.zeros(int(Sq[qq]) * 128, dtype=np.int16)
                   for qq in range(QS)]
        dstloc = np.full((TOTCH * 128,), PADVAL, dtype=np.float32)
        for qq in range(QS):
            mq = q_o == qq
            iq, dlq, trq = i_o[mq], dl_o[mq], tr_o[mq]
            blk_lo = np.searchsorted(iq, np.arange(NB))
            blk_hi = np.searchsorted(iq, np.arange(NB) + 1)
            for i in range(NB):
                a, b = int(blk_lo[i]), int(blk_hi[i])
                n_e = b - a
                if n_e == 0:
                    continue
                base = int(start_q[i, qq]) * 128
                idx16_q[qq][base:base + n_e] = (
                    trq[a:b] - qq * QROWS).astype(np.int16)
                gbase = _g_chunk_base(i, qq, K, start_q) * 128
                dstloc[gbase:gbase + n_e] = dlq[a:b].astype(np.float32)

        idx16_t = []
        for qq in range(QS):
            arr = np.zeros((P, nbatch_q[qq] * 256), dtype=np.int16)
            for b in range(nbatch_q[qq]):
                c0 = b * BATCH_CHUNKS
                nch = min(BATCH_CHUNKS, int(Sq[qq]) - c0)
                lin = idx16_q[qq][c0 * 128:(c0 + nch) * 128]
                wrapped = lin.reshape(-1, 16).T      # [16, nch*8]
                for grp in range(8):
                    arr[grp * 16:(grp + 1) * 16,
                        b * 256:b * 256 + nch * 8] = wrapped
            idx16_t.append(arr)

        dstloc_t = np.ascontiguousarray(dstloc.reshape(TOTCH, 128).T)

        xs = np.zeros((NSH, F), dtype=np.float32)
        base = c * NSH_RAW
        for i in range(NB):
            rb = int(perm_blocks[c, i])
            lo, hi = rb * 128, min(rb * 128 + 128, NSH_RAW)
            if hi > lo:
                xs[i * 128:i * 128 + (hi - lo)] = x[base + lo:base + hi]

        inp = {
            "xT": np.ascontiguousarray(xs.T),
            "W_in": np.asarray(W, dtype=np.float32),
            "attA": attA,
            "gbb": gbb,
            "dstloc16": dstloc_t,
        }
        for qq in range(QS):
            inp[f"idx16_{qq}"] = idx16_t[qq]
        per_core.append(inp)

    meta = dict(K=K, Sq=Sq, start_q=start_q, nbatch_q=nbatch_q,
                TOTCH=TOTCH, perm_blocks=perm_blocks)
    return per_core, meta


def _split_waits(nc, mybir, keep=1):
    """Walrus in this toolchain accepts at most one sem-wait on DMA/CTRL
    pseudo instructions; hoist excess waits onto InstEventSemaphore."""
    for f in nc.m.functions:
        for bb in f.blocks:
            new = []
            for ins in bb.instructions:
                si = ins.sync_info
                if si is not None and si.on_wait and len(si.on_wait) > keep:
                    for j, wcond in enumerate(list(si.on_wait)[:-keep]):
                        w = mybir.InstEventSemaphore(
                            name=f"{ins.name}-ws{j}", ins=[], outs=[])
                        w.engine = ins.engine
                        w.sync_info = mybir.SyncInfo(
                            on_wait=[wcond], on_update=[])
                        new.append(w)
                    ins.sync_info = mybir.SyncInfo(
                        on_wait=list(si.on_wait)[-keep:],
                        on_update=list(si.on_update))
                new.append(ins)
            bb.instructions[:] = new


def _build_program(meta, has_bias):
    import concourse.bass as bass
    import concourse.mybir as mybir
    import concourse.tile as tile
    from concourse.masks import make_identity
    from concourse.library_config import mlp as mlp_lib
    from concourse.library_overlay import lower_extended_insts

    K = meta["K"]; Sq = meta["Sq"]; start_q = meta["start_q"]
    nbatch_q = meta["nbatch_q"]; TOTCH = meta["TOTCH"]
    f16 = mybir.dt.float16
    f32 = mybir.dt.float32
    i16 = mybir.dt.int16
    AF = mybir.ActivationFunctionType
    OP = mybir.AluOpType

    nc = bass.Bass(num_devices=NCORES, num_swdge_queues=QS)
    xT = nc.dram_tensor("xT", [F, NSH], f32, kind="ExternalInput")
    W_in = nc.dram_tensor("W_in", [F, HC], f32, kind="ExternalInput")
    attA_in = nc.dram_tensor("attA", [F, 4], f32, kind="ExternalInput")
    gbb_in = nc.dram_tensor("gbb", [1, 3 * HC], f32, kind="ExternalInput")
    dstloc_in = nc.dram_tensor("dstloc16", [P, TOTCH], f32,
                               kind="ExternalInput")
    idx_in = [nc.dram_tensor(f"idx16_{q}", [P, nbatch_q[q] * 256], i16,
                             kind="ExternalInput") for q in range(QS)]
    out_dram = nc.dram_tensor("out_shard", [NSH, HC], f32,
                              kind="ExternalOutput")
    import os as _os
    _dbg = bool(_os.environ.get("KERNEL_DEBUG"))
    if _dbg:
        dbg_pre = nc.dram_tensor("dbg_pre", [NSH, HC], f32,
                                 kind="ExternalOutput")
        dbg_st = nc.dram_tensor("dbg_st", [1, 2 * HC], f32,
                                kind="ExternalOutput")
        dbg_g = nc.dram_tensor("dbg_g", [P, P], f32, kind="ExternalOutput")
        dbg_eq = nc.dram_tensor("dbg_eq", [P, P], f32, kind="ExternalOutput")
        dbg_ev = nc.dram_tensor("dbg_ev", [P, 8], f32, kind="ExternalOutput")
        dbg_agg = nc.dram_tensor("dbg_agg", [P, HC + 2], f32,
                                 kind="ExternalOutput")

    with tile.TileContext(nc) as tc:
        with tc.tile_pool(name="cst", bufs=1) as cst, \
             tc.tile_pool(name="sb", bufs=2) as sb, \
             tc.tile_pool(name="ps", bufs=1, space="PSUM") as psp, \
             tc.tile_pool(name="dram", bufs=1, space="DRAM") as dram:

            # ---------------- constants ----------------
            ident = cst.tile([P, P], f16)
            make_identity(nc, ident[:])
            iota_i = cst.tile([P, P], mybir.dt.int32)
            nc.gpsimd.iota(iota_i[:], pattern=[[1, P]], channel_multiplier=0)
            iota16 = cst.tile([P, P], f16)
            nc.vector.tensor_copy(iota16[:], iota_i[:])
            ones16 = cst.tile([P, 1], f16)
            nc.vector.memset(ones16[:], 1.0)
            onescol = cst.tile([P, 1], f32)
            nc.vector.memset(onescol[:], 1.0)
            ones_row = cst.tile([1, P], f32)
            nc.vector.memset(ones_row[:], 1.0)
            W_sb = cst.tile([F, HC], f32)
            nc.sync.dma_start(W_sb[:], W_in[:])
            attA_f = cst.tile([F, 4], f32)
            nc.sync.dma_start(attA_f[:], attA_in[:])
            attA_sb = cst.tile([F, 4], f16)
            nc.vector.tensor_copy(attA_sb[:], attA_f[:])
            gbb_sb = cst.tile([1, 3 * HC], f32)
            nc.sync.dma_start(gbb_sb[:], gbb_in[:])
            dstloc_sb = cst.tile([P, TOTCH], f32)
            nc.sync.dma_start(dstloc_sb[:], dstloc_in[:])
            idx_sb = []
            for q in range(QS):
                t = cst.tile([P, nbatch_q[q] * 256], i16, name=f"idxsb{q}")
                nc.sync.dma_start(t[:], idx_in[q][:])
                idx_sb.append(t)
            out_acc = cst.tile([P, NB * HC], f32)

            nc.gpsimd.load_library(mlp_lib)

            # ---------------- phase 0: h table ----------------
            h_shard = dram.tile([NSH, HC], f16)
            h_full = dram.tile([NTAB, HC], f16)
            for i in range(NB):
                xt_t = sb.tile([F, P], f32, tag="xt", bufs=3)
                nc.sync.dma_start(xt_t[:], xT[:, i * 128:(i + 1) * 128])
                h_ps = psp.tile([P, HC], f32, tag="tp", bufs=2)
                nc.tensor.matmul(h_ps[:], lhsT=xt_t[:], rhs=W_sb[:],
                                 start=True, stop=True)
                h_sb = sb.tile([P, HC], f16, tag="hsb", bufs=3)
                nc.scalar.copy(h_sb[:], h_ps[:])
                nc.sync.dma_start(h_shard[i * 128:(i + 1) * 128, :], h_sb[:])
            nc.gpsimd.collective_compute(
                "AllGather", OP.bypass,
                replica_groups=[list(range(NCORES))],
                ins=[h_shard[:].opt()], outs=[h_full[:].opt()])

            # ---------------- gathers ----------------
            gtiles = {}
            nidx_regs = {}

            def reg_for(v):
                if v not in nidx_regs:
                    nidx_regs[v] = nc.gpsimd.to_reg(v)
                return nidx_regs[v]

            def issue_gather(q, b):
                c0 = b * BATCH_CHUNKS
                nch = min(BATCH_CHUNKS, int(Sq[q]) - c0)
                gt = sb.tile([P, BATCH_CHUNKS * HC], f16, tag=f"g{q}",
                             bufs=2, name=f"g{q}_{b}")
                nc.gpsimd.dma_gather(
                    out_ap=gt[:, 0:nch * HC].rearrange(
                        "p (k d) -> p k d", d=HC),
                    in_ap=h_full[q * QROWS:(q + 1) * QROWS, :],
                    idxs_ap=idx_sb[q][:, b * 256:b * 256 + nch * 8],
                    num_idxs=nch * 128,
                    num_idxs_reg=reg_for(nch * 128),
                    elem_size=HC,
                    single_packet=False,
                    queue_num=q)
                gtiles[(q, b)] = gt

            stats_ps = psp.tile([1, 2 * HC], f32, tag="stats", bufs=1)
            nc.vector.memset(stats_ps[:], 0.0)

            dbg_state = {"done": False}

            def chunk_pipeline(g_ap, dl_col, adst_sb, agg_ps, first, last,
                               self_chunk, dbg_this=False):
                gT_ps = psp.tile([P, P], f16, tag="tp", bufs=2)
                nc.tensor.transpose(gT_ps[:], g_ap, ident[:])
                gT_sb = sb.tile([P, P], f16, tag="gT", bufs=3)
                nc.scalar.copy(gT_sb[:], gT_ps[:])
                sc_ps = psp.tile([P, 4], f32, tag="sc", bufs=1)
                ncols = 4 if self_chunk else 2
                nc.tensor.matmul(sc_ps[:, 0:ncols], lhsT=gT_sb[:],
                                 rhs=attA_sb[:, 0:ncols], start=True,
                                 stop=True)
                adst_new = None
                if self_chunk:
                    eq_ap = ident[:]
                    adst_new = sb.tile([P, 2], f16, tag="adstb", bufs=2)
                    nc.vector.tensor_copy(adst_new[:], sc_ps[:, 2:4])
                    adx = adst_new
                else:
                    eq = sb.tile([P, P], f16, tag="eq", bufs=3)
                    nc.vector.tensor_scalar(out=eq[:], in0=iota16[:],
                                            scalar1=dl_col, scalar2=None,
                                            op0=OP.is_equal)
                    eq_ap = eq[:]
                    mT_ps = psp.tile([P, P], f16, tag="mT", bufs=2)
                    nc.tensor.transpose(mT_ps[:], eq[:], ident[:])
                    mT_sb = sb.tile([P, P], f16, tag="mT", bufs=3)
                    nc.vector.tensor_copy(mT_sb[:], mT_ps[:])
                    nc.tensor.matmul(sc_ps[:, 2:4], lhsT=mT_sb[:],
                                     rhs=adst_sb[:], start=True, stop=True)
                    adx = sb.tile([P, 2], f16, tag="adx", bufs=3)
                    nc.scalar.copy(adx[:], sc_ps[:, 2:4])
                esc = sb.tile([P, 2], f32, tag="esc", bufs=3)
                nc.vector.tensor_tensor(out=esc[:], in0=sc_ps[:, 0:2],
                                        in1=adx[:], op=OP.add)
                t02 = sb.tile([P, 2], f32, tag="t02", bufs=3)
                nc.vector.tensor_scalar(out=t02[:], in0=esc[:],
                                        scalar1=NEG_SLOPE, scalar2=None,
                                        op0=OP.mult)
                lr = sb.tile([P, 2], f32, tag="lr", bufs=3)
                nc.vector.tensor_tensor(out=lr[:], in0=t02[:], in1=esc[:],
                                        op=OP.max)
                expv = sb.tile([P, 2], f32, tag="expv", bufs=3)
                nc.scalar.activation(expv[:], lr[:], AF.Exp)
                if _dbg and dbg_this and not dbg_state["done"]:
                    dbg_state["done"] = True
                    tg = sb.tile([P, P], f32, tag="dbgg", bufs=1)
                    nc.vector.tensor_copy(tg[:], g_ap)
                    nc.sync.dma_start(dbg_g[:], tg[:])
                    te = sb.tile([P, P], f32, tag="dbge", bufs=1)
                    nc.vector.tensor_copy(te[:], eq_ap)
                    nc.sync.dma_start(dbg_eq[:], te[:])
                    tv = sb.tile([P, 8], f32, tag="dbgv", bufs=1)
                    nc.vector.tensor_copy(tv[:, 0:2], expv[:])
                    nc.vector.tensor_copy(tv[:, 2:4], esc[:])
                    nc.vector.tensor_copy(tv[:, 4:6], sc_ps[:, 0:2])
                    nc.vector.tensor_copy(tv[:, 6:8], adx[:])
                    nc.sync.dma_start(dbg_ev[:], tv[:])
                for h in range(H):
                    em = sb.tile([P, P], f16, tag=f"em{h}", bufs=3)
                    nc.vector.tensor_scalar(out=em[:], in0=eq_ap,
                                            scalar1=expv[:, h:h + 1],
                                            scalar2=None, op0=OP.mult)
                    nc.tensor.matmul(agg_ps[:, C * h:C * (h + 1)],
                                     lhsT=em[:],
                                     rhs=g_ap[:, C * h:C * (h + 1)],
                                     start=False, stop=last)
                    nc.tensor.matmul(agg_ps[:, HC + h:HC + h + 1],
                                     lhsT=em[:], rhs=ones16[:],
                                     start=False, stop=last)
                return adst_new

            if has_bias:
                bias_ps = psp.tile([P, HC], f32, tag="tp", bufs=2)
                nc.tensor.matmul(bias_ps[:], lhsT=ones_row[:],
                                 rhs=gbb_sb[:, 2 * HC:3 * HC], start=True, stop=True)
                bias_bc = cst.tile([P, HC], f32)
                nc.vector.tensor_copy(bias_bc[:], bias_ps[:])

            for i in range(NB):
                agg_ps = psp.tile([P, HC + 2], f32, tag="agg", bufs=2,
                                  name=f"agg{i}")
                nc.vector.memset(agg_ps[:], 0.0)
                nchunks = 1 + int(K[i].sum())
                gself = sb.tile([P, HC], f16, tag="gself", bufs=2)
                nc.sync.dma_start(gself[:],
                                  h_shard[i * 128:(i + 1) * 128, :])
                adst_blk = chunk_pipeline(gself[:], None, None, agg_ps,
                                          True, nchunks == 1, True)
                done = 1
                for q in range(QS):
                    for k in range(int(K[i, q])):
                        sq_chunk = int(start_q[i, q]) + k
                        b = sq_chunk // BATCH_CHUNKS
                        j = sq_chunk % BATCH_CHUNKS
                        if (q, b) not in gtiles:
                            issue_gather(q, b)
                        gt = gtiles[(q, b)]
                        gcol = _g_chunk_base(i, q, K, start_q) + k
                        done += 1
                        chunk_pipeline(gt[:, j * HC:(j + 1) * HC],
                                       dstloc_sb[:, gcol:gcol + 1],
                                       adst_blk, agg_ps, False,
                                       done == nchunks, False,
                                       dbg_this=(i == 0 and done == 2))

                if _dbg and i == 0:
                    ta = sb.tile([P, HC + 2], f32, tag="dbga", bufs=1)
                    nc.vector.tensor_copy(ta[:], agg_ps[:])
                    nc.sync.dma_start(dbg_agg[:], ta[:])
                recip = sb.tile([P, 2], f32, tag="recip", bufs=2)
                nc.vector.reciprocal(recip[:], agg_ps[:, HC:HC + 2])
                oslice = out_acc[:, i * HC:(i + 1) * HC]
                for h in range(H):
                    if has_bias:
                        tmp = sb.tile([P, C], f32, tag="tmpb", bufs=2)
                        nc.vector.tensor_scalar(
                            out=tmp[:], in0=agg_ps[:, C * h:C * (h + 1)],
                            scalar1=recip[:, h:h + 1], scalar2=None,
                            op0=OP.mult)
                        nc.vector.tensor_tensor(
                            out=tmp[:], in0=tmp[:],
                            in1=bias_bc[:, C * h:C * (h + 1)], op=OP.add)
                        nc.vector.tensor_scalar(
                            out=oslice[:, C * h:C * (h + 1)], in0=tmp[:],
                            scalar1=0.0, scalar2=None, op0=OP.max)
                    else:
                        nc.vector.tensor_scalar(
                            out=oslice[:, C * h:C * (h + 1)],
                            in0=agg_ps[:, C * h:C * (h + 1)],
                            scalar1=recip[:, h:h + 1], scalar2=0.0,
                            op0=OP.mult, op1=OP.max)
                sq_t = sb.tile([P, HC], f32, tag="sq", bufs=2)
                nc.vector.tensor_tensor(out=sq_t[:], in0=oslice, in1=oslice,
                                        op=OP.mult)
                nc.tensor.matmul(stats_ps[:, 0:HC], lhsT=onescol[:],
                                 rhs=oslice, start=False,
                                 stop=(i == NB - 1))
                nc.tensor.matmul(stats_ps[:, HC:2 * HC], lhsT=onescol[:],
                                 rhs=sq_t[:], start=False,
                                 stop=(i == NB - 1))

            # ---------------- BN epilogue ----------------
            st_sb = sb.tile([1, 2 * HC], f32, tag="st", bufs=1)
            nc.vector.tensor_copy(st_sb[:], stats_ps[:])
            st_loc = dram.tile([1, 2 * HC], f32)
            st_glob = dram.tile([1, 2 * HC], f32)
            nc.sync.dma_start(st_loc[:], st_sb[:])
            nc.gpsimd.collective_compute(
                "AllReduce", OP.add,
                replica_groups=[list(range(NCORES))],
                ins=[st_loc[:].opt()], outs=[st_glob[:].opt()])
            st_g = sb.tile([1, 2 * HC], f32, tag="stg", bufs=1)
            nc.sync.dma_start(st_g[:], st_glob[:])
            if _dbg:
                nc.sync.dma_start(dbg_st[:], st_g[:])
                for i in range(NB):
                    nc.sync.dma_start(
                        dbg_pre[i * 128:(i + 1) * 128, :],
                        out_acc[:, i * HC:(i + 1) * HC])

            sc2 = sb.tile([1, 2 * HC], f32, tag="sc2", bufs=1)
            # mean in sc2[:, 0:HC] (temp), E[x^2] in temp row usage below
            mrow = sb.tile([1, HC], f32, tag="mrow", bufs=1)
            nc.vector.tensor_scalar(out=mrow[:], in0=st_g[:, 0:HC],
                                    scalar1=1.0 / N, scalar2=None,
                                    op0=OP.mult)
            vrow = sb.tile([1, HC], f32, tag="vrow", bufs=1)
            nc.vector.tensor_scalar(out=vrow[:], in0=st_g[:, HC:2 * HC],
                                    scalar1=1.0 / N, scalar2=None,
                                    op0=OP.mult)
            m2 = sb.tile([1, HC], f32, tag="m2", bufs=1)
            nc.vector.tensor_tensor(out=m2[:], in0=mrow[:], in1=mrow[:],
                                    op=OP.mult)
            nc.vector.tensor_tensor(out=vrow[:], in0=vrow[:], in1=m2[:],
                                    op=OP.subtract)
            nc.vector.tensor_scalar(out=vrow[:], in0=vrow[:],
                                    scalar1=BN_EPS, scalar2=None, op0=OP.add)
            rinv = sb.tile([1, HC], f32, tag="rinv", bufs=1)
            nc.vector.reciprocal(rinv[:], vrow[:])
            rstd = sb.tile([1, HC], f32, tag="rstd", bufs=1)
            nc.scalar.activation(rstd[:], rinv[:], AF.Sqrt)
            # scale = gamma * rstd ; shift = beta - mean*scale
            nc.vector.tensor_tensor(out=sc2[:, 0:HC], in0=gbb_sb[:, 0:HC],
                                    in1=rstd[:], op=OP.mult)
            msc = sb.tile([1, HC], f32, tag="msc", bufs=1)
            nc.vector.tensor_tensor(out=msc[:], in0=mrow[:],
                                    in1=sc2[:, 0:HC], op=OP.mult)
            nc.vector.tensor_tensor(out=sc2[:, HC:2 * HC], in0=gbb_sb[:, HC:2 * HC],
                                    in1=msc[:], op=OP.subtract)
            bc_ps = psp.tile([P, 2 * HC], f32, tag="mT", bufs=2)
            nc.tensor.matmul(bc_ps[:], lhsT=ones_row[:], rhs=sc2[:],
                             start=True, stop=True)
            bc_sb = sb.tile([P, 2 * HC], f32, tag="bc", bufs=1)
            nc.vector.tensor_copy(bc_sb[:], bc_ps[:])

            for i in range(NB):
                fin = sb.tile([P, HC], f32, tag="fin", bufs=3)
                nc.vector.tensor_tensor(out=fin[:],
                                        in0=out_acc[:, i * HC:(i + 1) * HC],
                                        in1=bc_sb[:, 0:HC], op=OP.mult)
                nc.vector.tensor_tensor(out=fin[:], in0=fin[:],
                                        in1=bc_sb[:, HC:2 * HC], op=OP.add)
                nc.sync.dma_start(out_dram[i * 128:(i + 1) * 128, :], fin[:])

    from concourse.library_overlay import lower_extended_insts as _lei
    _lei(nc)
    _split_waits(nc, mybir)
    return nc


_CACHE = {}


def kernel(**inputs):
    x = inputs["x"]
    edge_index = inputs["edge_index"]
    W = inputs["W"]
    att_src = inputs["att_src"]
    att_dst = inputs["att_dst"]
    bias = inputs["bias"]
    gamma = inputs["gamma"]
    beta = inputs["beta"]

    per_core, meta = _host_prep(x, edge_index, W, att_src, att_dst,
                                bias, gamma, beta)
    has_bias = bool(np.any(np.asarray(bias) != 0))

    key = ("prog", tuple(meta["K"].reshape(-1).tolist()), has_bias)
    if key in _CACHE:
        nc = _CACHE[key]
    else:
        nc = _build_program(meta, has_bias)
        _CACHE[key] = nc

    from concourse.bass_utils import run_bass_kernel_spmd
    res = run_bass_kernel_spmd(nc, per_core, core_ids=list(range(NCORES)))

    out = np.zeros((N, HC), dtype=np.float32)
    perm_blocks = meta["perm_blocks"]
    for c in range(NCORES):
        shard = res.results[c]["out_shard"]          # [NSH, HC] block-permuted
        base = c * NSH_RAW
        for i in range(NB):
            rb = int(perm_blocks[c, i])
            lo, hi = rb * 128, min(rb * 128 + 128, NSH_RAW)
            if hi > lo:
                out[base + lo:base + hi] = shard[i * 128:i * 128 + (hi - lo)]
    return out



# revision 28
# speedup vs baseline: 1.4172x; 1.4172x over previous
"""GAT (2-head, 64-ch) + BatchNorm message passing on 8 Trainium2 cores.

Dst-node graph parallel: 12500 dst nodes/core (98 blocks x 128, in-degree
sorted so per-block edge counts are uniform across cores). Phase 0 computes
h_aug = x @ [W | W@att_src^T | W@att_dst^T] per shard into 512-byte table
rows [h(128f16) | a_src(2) | a_dst(2) | pad]; the table is AllGathered in 4
rank-quarter pieces so quartile-q gathers can start as soon as piece q
lands. Edges are routed to the dst core, bucketed per (dst block, src
quartile) padded to 128-edge chunks (uniform chunk grid across cores), and
gathered 8 chunks (1024 rows) per dma_gather on 4 SWDGE queues with
prefetch. Per chunk: a_dst expand via one PE matmul with a host-uploaded
transposed one-hot (eqT); esc/leaky-relu/exp batched per 8-chunk batch;
w-scaled values built per head on DVE/ACT; one PE matmul per chunk
accumulates numerator and denominator into the block PSUM. Block finalize
adds the self-loop (scores straight from phase-0 columns), normalizes,
applies ReLU, and accumulates BN stats; stats are AllReduced and the BN
affine applied in a final pass.
"""
import sys
sys.path.insert(0, "/opt/trn_rl_repo")
import numpy as np

N = 100_000
F = 128
H = 2
C = 64
HC = H * C
NEG_SLOPE = 0.2
BN_EPS = 1e-5
NCORES = 8
NSH_RAW = 12_500
NSH = 12_544            # 98 * 128
NB = NSH // 128         # 98
QS = 4
# block-aligned shard quarters (ranks); quartile tables are 8x these rows
QSH_P = [3200, 3200, 3072, 3072]
PS_P = [0, 3200, 6400, 9472, 12544]
P = 128
ROWW = 256              # table row width in f16 (512 B)
BCH = 8                 # chunks per gather batch (1024 rows)
BROWS = BCH * P
PADVAL = 200.0
PREFETCH = 2


def _host_prep(x, edge_index, W, att_src, att_dst, bias, gamma, beta):
    src = np.asarray(edge_index[0]).astype(np.int64)
    dst = np.asarray(edge_index[1]).astype(np.int64)
    x = np.asarray(x, dtype=np.float32)
    W = np.asarray(W, dtype=np.float32)
    att_src = np.asarray(att_src, dtype=np.float32)
    att_dst = np.asarray(att_dst, dtype=np.float32)

    W_aug = np.zeros((F, 132), dtype=np.float32)
    W_aug[:, 0:HC] = W
    W_aug[:, HC:HC + 2] = np.einsum(
        "fhc,hc->fh", W.reshape(F, H, C), att_src)
    W_aug[:, HC + 2:HC + 4] = np.einsum(
        "fhc,hc->fh", W.reshape(F, H, C), att_dst)

    gbb = np.zeros((1, 3 * HC), dtype=np.float32)
    gbb[0, 0:HC] = np.asarray(gamma, dtype=np.float32).reshape(-1)
    gbb[0, HC:2 * HC] = np.asarray(beta, dtype=np.float32).reshape(-1)
    gbb[0, 2 * HC:] = np.asarray(bias, dtype=np.float32).reshape(-1)

    # per-core in-degree rank (degree-sorted blocks)
    orders, ranks = [], []
    for c in range(NCORES):
        m = (dst // NSH_RAW) == c
        d_loc = dst[m] - c * NSH_RAW
        deg = np.bincount(d_loc, minlength=NSH_RAW)
        order = np.argsort(-deg, kind="stable")
        rank = np.empty(NSH_RAW, dtype=np.int64)
        rank[order] = np.arange(NSH_RAW)
        orders.append(order)
        ranks.append(rank)

    ps = np.asarray(PS_P, dtype=np.int64)
    qsh = np.asarray(QSH_P, dtype=np.int64)

    # per-core edge bucketing by (dst block, src quartile)
    core_ed = []
    cnts = np.zeros((NCORES, NB, QS), dtype=np.int64)
    for c in range(NCORES):
        m = (dst // NSH_RAW) == c
        s_c = src[m]
        r_d = ranks[c][dst[m] - c * NSH_RAW]
        w = r_d // 128
        j = r_d % 128
        cs = s_c // NSH_RAW
        r_s_local = np.empty(len(s_c), dtype=np.int64)
        for c2 in range(NCORES):
            mm = cs == c2
            r_s_local[mm] = ranks[c2][s_c[mm] - c2 * NSH_RAW]
        q = (np.searchsorted(ps, r_s_local, side="right") - 1).astype(np.int64)
        idx16 = cs * qsh[q] + (r_s_local - ps[q])
        core_ed.append((w, j, q, idx16))
        np.add.at(cnts[c], (w, q), 1)

    K = ((cnts.max(axis=0) + 127) // 128).astype(np.int64)   # [NB, QS]
    SK_q = K.sum(axis=0)                                     # chunks/stream
    TOTCH = int(SK_q.sum())
    nbatch_q = [int((SK_q[q] + BCH - 1) // BCH) for q in range(QS)]
    base_wq = np.zeros((NB, QS), dtype=np.int64)             # chunk base of (w,q)
    for q in range(QS):
        base_wq[1:, q] = np.cumsum(K[:-1, q])
    off_q = np.zeros(QS, dtype=np.int64)                     # stream col offset
    off_q[1:] = np.cumsum(SK_q[:-1])

    per_core = []
    for c in range(NCORES):
        w, j, q, idx16 = core_ed[c]
        ordv = np.lexsort((idx16, j, w + NB * q))
        wq, jq, qq, iq = w[ordv], j[ordv], q[ordv], idx16[ordv]

        idx_streams = [np.zeros(nbatch_q[s] * BROWS, dtype=np.int16)
                       for s in range(QS)]
        dl = np.full((TOTCH * 128,), PADVAL, dtype=np.float32)
        eqT = np.zeros((128, TOTCH * 128), dtype=np.float16)
        for s in range(QS):
            ms = qq == s
            ws, js, is_ = wq[ms], jq[ms], iq[ms]
            blo = np.searchsorted(ws, np.arange(NB))
            bhi = np.searchsorted(ws, np.arange(NB) + 1)
            for wv in range(NB):
                a, b = int(blo[wv]), int(bhi[wv])
                ne = b - a
                if ne == 0:
                    continue
                sbase = int(base_wq[wv, s]) * 128          # row in stream s
                idx_streams[s][sbase:sbase + ne] = is_[a:b].astype(np.int16)
                gch = (int(off_q[s]) + int(base_wq[wv, s])) * 128
                dl[gch:gch + ne] = js[a:b].astype(np.float32)
                eqT[js[a:b], gch + np.arange(ne)] = 1.0

        inp = {
            "xT": None,          # filled below
            "W_aug": W_aug,
            "gbb": gbb,
            "dl": np.ascontiguousarray(
                dl.reshape(TOTCH, 128).T).astype(np.float32),
            "eqT": eqT,
        }
        for s in range(QS):
            lin = idx_streams[s]
            wrapped = lin.reshape(-1, 16).T                  # [16, nb*64]
            arr = np.zeros((P, nbatch_q[s] * (BROWS // 16)), dtype=np.int16)
            for grp in range(8):
                arr[grp * 16:(grp + 1) * 16, :] = wrapped
            inp[f"idx16_{s}"] = arr

        xs = np.zeros((NSH, F), dtype=np.float32)
        xs[:NSH_RAW] = x[c * NSH_RAW:(c + 1) * NSH_RAW][orders[c]]
        inp["xT"] = np.ascontiguousarray(xs.T)               # [F, NSH]
        per_core.append(inp)

    meta = dict(K=K, SK_q=SK_q, TOTCH=TOTCH, nbatch_q=nbatch_q,
                base_wq=base_wq, off_q=off_q, orders=orders)
    return per_core, meta


def _split_waits(nc, mybir, keep=1):
    """Walrus accepts at most one sem-wait on DMA/CTRL pseudo instructions;
    hoist excess waits onto InstEventSemaphore."""
    for f in nc.m.functions:
        for bb in f.blocks:
            new = []
            for ins in bb.instructions:
                si = ins.sync_info
                if si is not None and si.on_wait and len(si.on_wait) > keep:
                    for jj, wcond in enumerate(list(si.on_wait)[:-keep]):
                        w = mybir.InstEventSemaphore(
                            name=f"{ins.name}-ws{jj}", ins=[], outs=[])
                        w.engine = ins.engine
                        w.sync_info = mybir.SyncInfo(
                            on_wait=[wcond], on_update=[])
                        new.append(w)
                    ins.sync_info = mybir.SyncInfo(
                        on_wait=list(si.on_wait)[-keep:],
                        on_update=list(si.on_update))
                new.append(ins)
            bb.instructions[:] = new


def _build_program(meta, has_bias):
    import concourse.bass as bass
    import concourse.mybir as mybir
    import concourse.tile as tile
    from concourse.masks import make_identity
    from concourse.library_config import mlp as mlp_lib
    from concourse.library_overlay import lower_extended_insts

    K = meta["K"]; SK_q = meta["SK_q"]; TOTCH = meta["TOTCH"]
    nbatch_q = meta["nbatch_q"]; base_wq = meta["base_wq"]
    off_q = meta["off_q"]
    f16 = mybir.dt.float16
    f32 = mybir.dt.float32
    i16 = mybir.dt.int16
    AF = mybir.ActivationFunctionType
    OP = mybir.AluOpType

    nc = bass.Bass(num_devices=NCORES, num_swdge_queues=QS)
    xT_in = nc.dram_tensor("xT", [F, NSH], f32, kind="ExternalInput")
    W_in = nc.dram_tensor("W_aug", [F, 132], f32, kind="ExternalInput")
    gbb_in = nc.dram_tensor("gbb", [1, 3 * HC], f32, kind="ExternalInput")
    dl_in = nc.dram_tensor("dl", [P, TOTCH], f32, kind="ExternalInput")
    eqT_in = nc.dram_tensor("eqT", [P, TOTCH * 128], f16,
                            kind="ExternalInput")
    idx_in = [nc.dram_tensor(f"idx16_{q}", [P, nbatch_q[q] * (BROWS // 16)],
                             i16, kind="ExternalInput") for q in range(QS)]
    out_dram = nc.dram_tensor("out_shard", [NSH, HC], f32,
                              kind="ExternalOutput")
    import os as _os
    _dbg = bool(_os.environ.get("KERNEL_DEBUG"))
    if _dbg:
        dbg_g = nc.dram_tensor("dbg_g", [P, BCH * ROWW], f32,
                               kind="ExternalOutput")
        dbg_esc = nc.dram_tensor("dbg_esc", [P, 2 * BCH], f32,
                                 kind="ExternalOutput")
        dbg_rhs = nc.dram_tensor("dbg_rhs", [P, BCH * 130], f32,
                                 kind="ExternalOutput")
        dbg_pre = nc.dram_tensor("dbg_pre", [NSH, HC], f32,
                                 kind="ExternalOutput")
        dbg_hsh = nc.dram_tensor("dbg_hsh", [P, ROWW], f32,
                                 kind="ExternalOutput")
        dbg_hfu = nc.dram_tensor("dbg_hfu", [P, ROWW], f32,
                                 kind="ExternalOutput")

    # chunk -> block map per stream
    blockof = [np.repeat(np.arange(NB), K[:, q]) for q in range(QS)]
    # first/last chunk of each block (global over the 4 streams' chunklists)
    remaining0 = K.sum(axis=1)

    with tile.TileContext(nc) as tc:
        with tc.tile_pool(name="cst", bufs=1) as cst, \
             tc.tile_pool(name="sb", bufs=2) as sb, \
             tc.tile_pool(name="ps", bufs=1, space="PSUM") as psp, \
             tc.tile_pool(name="dram", bufs=1, space="DRAM") as dram:

            ident = cst.tile([P, P], f16)
            make_identity(nc, ident[:])
            iota_i = cst.tile([P, P], mybir.dt.int32)
            nc.gpsimd.iota(iota_i[:], pattern=[[1, P]], channel_multiplier=0)
            iota16 = cst.tile([P, P], f16)
            nc.vector.tensor_copy(iota16[:], iota_i[:])
            ones16 = cst.tile([P, 1], f16)
            nc.vector.memset(ones16[:], 1.0)
            ones_row = cst.tile([1, P], f32)
            nc.vector.memset(ones_row[:], 1.0)
            W_sb = cst.tile([F, 132], f32)
            nc.sync.dma_start(W_sb[:], W_in[:])
            gbb_sb = cst.tile([1, 3 * HC], f32)
            nc.sync.dma_start(gbb_sb[:], gbb_in[:])
            dl_sb = cst.tile([P, TOTCH], f32)
            nc.sync.dma_start(dl_sb[:], dl_in[:])
            idx_sb = []
            for q in range(QS):
                t = cst.tile([P, nbatch_q[q] * (BROWS // 16)], i16,
                             name=f"idxsb{q}")
                nc.sync.dma_start(t[:], idx_in[q][:])
                idx_sb.append(t)
            sc_acc = cst.tile([P, NB * 4], f16)
            out_acc = cst.tile([P, NB * HC], f16)

            nc.gpsimd.load_library(mlp_lib)

            # ---------------- phase 0: augmented h table ----------------
            h_shard_p = [dram.tile([QSH_P[p], ROWW], f16, name=f"hsh{p}")
                         for p in range(QS)]
            h_full_p = [dram.tile([NCORES * QSH_P[p], ROWW], f16,
                                  name=f"hfu{p}") for p in range(QS)]
            ag_after = {(PS_P[p + 1] - 1) // 128: p for p in range(QS)}

            nidx_regs = {}

            def reg_for(v):
                if v not in nidx_regs:
                    nidx_regs[v] = nc.gpsimd.to_reg(v)
                return nidx_regs[v]

            gtiles = {}

            def issue_gather(q, b):
                gt = sb.tile([P, BCH * ROWW], f16, tag=f"g{q}", bufs=3,
                             name=f"g{q}_{b}")
                nc.gpsimd.dma_gather(
                    out_ap=gt[:].rearrange("p (k d) -> p k d", d=ROWW),
                    in_ap=h_full_p[q][:, :],
                    idxs_ap=idx_sb[q][:, b * (BROWS // 16):
                                      (b + 1) * (BROWS // 16)],
                    num_idxs=BROWS,
                    num_idxs_reg=reg_for(BROWS),
                    elem_size=ROWW,
                    single_packet=False,
                    queue_num=q)
                gtiles[(q, b)] = gt

            next_issue = [0] * QS

            def prefetch(q, upto):
                while next_issue[q] <= min(upto, nbatch_q[q] - 1):
                    issue_gather(q, next_issue[q])
                    next_issue[q] += 1

            GRP = 14                       # blocks per phase-0 group
            for g in range(NB // GRP):
                xt = sb.tile([P, GRP * P], f32, tag="xt", bufs=2)
                nc.sync.dma_start(
                    xt[:], xT_in[:, g * GRP * 128:(g + 1) * GRP * 128])
                h_sb = sb.tile([P, GRP * ROWW], f16, tag="hsb", bufs=2)
                for v in range(GRP):
                    w = g * GRP + v
                    h_ps = psp.tile([P, 132], f32, tag="escp", bufs=2)
                    nc.tensor.matmul(h_ps[:], lhsT=xt[:, v * 128:
                                                      (v + 1) * 128],
                                     rhs=W_sb[:], start=True, stop=True)
                    nc.scalar.copy(
                        h_sb[:, v * ROWW:v * ROWW + 132], h_ps[:])
                    nc.vector.tensor_copy(sc_acc[:, 4 * w:4 * w + 4],
                                          h_ps[:, 128:132])
                # write group rows [1792g, 1792(g+1)) split by quarter piece
                r0, r1 = g * GRP * 128, (g + 1) * GRP * 128
                hv = h_sb[:].rearrange("p (v d) -> p v d", d=ROWW)
                for pc in range(QS):
                    a = max(r0, PS_P[pc])
                    bnd = min(r1, PS_P[pc + 1])
                    if a >= bnd:
                        continue
                    nc.sync.dma_start(
                        h_shard_p[pc][a - PS_P[pc]:bnd - PS_P[pc], :]
                        .rearrange("(v j) d -> j v d", j=P),
                        hv[:, (a - r0) // 128:(bnd - r0) // 128, :])
                for w in range(g * GRP, (g + 1) * GRP):
                    if w in ag_after:
                        p = ag_after[w]
                        nc.gpsimd.collective_compute(
                            "AllGather", OP.bypass,
                            replica_groups=[list(range(NCORES))],
                            ins=[h_shard_p[p][:].opt()],
                            outs=[h_full_p[p][:].opt()])
            for q in range(QS):
                prefetch(q, PREFETCH)

            # self-loop scores for all blocks at once
            sc_v = sc_acc[:].rearrange("p (w d) -> p w d", d=4)
            esc_s = cst.tile([P, NB * 2], f32)
            nc.vector.tensor_tensor(
                out=esc_s[:].rearrange("p (w d) -> p w d", d=2),
                in0=sc_v[:, :, 0:2], in1=sc_v[:, :, 2:4], op=OP.add)
            t02_s = cst.tile([P, NB * 2], f32)
            nc.vector.tensor_scalar(out=t02_s[:], in0=esc_s[:],
                                    scalar1=NEG_SLOPE, scalar2=None,
                                    op0=OP.mult)
            nc.vector.tensor_tensor(out=esc_s[:], in0=t02_s[:],
                                    in1=esc_s[:], op=OP.max)
            expv_s = cst.tile([P, NB * 2], f32)
            nc.scalar.activation(expv_s[:], esc_s[:], AF.Exp)

            if has_bias:
                bias_ps = psp.tile([P, HC], f32, tag="escp", bufs=2)
                nc.tensor.matmul(bias_ps[:], lhsT=ones_row[:],
                                 rhs=gbb_sb[:, 2 * HC:3 * HC],
                                 start=True, stop=True)
                bias_bc = cst.tile([P, HC], f32)
                nc.vector.tensor_copy(bias_bc[:], bias_ps[:])

            stats_ps = psp.tile([1, 2 * HC], f32, tag="stats", bufs=1)
            nc.vector.memset(stats_ps[:], 0.0)

            # ---------------- main loop (batch-major) ----------------
            agg_tiles = {}           # triple t -> psum tile [P, 3*130]
            triple_left = {}
            started = set()
            remaining = remaining0.copy()
            nfin = [0]

            def agg_slice(w):
                if w not in agg_tiles:
                    agg_tiles[w] = psp.tile([P, HC + 2], f32,
                                            tag="agg", bufs=5,
                                            name=f"agg{w}")
                return agg_tiles[w], 0

            def finalize(w):
                gself = sb.tile([P, HC], f16, tag="gself", bufs=3)
                p0 = next(p for p in range(QS)
                          if PS_P[p] <= w * 128 < PS_P[p + 1])
                r0 = w * 128 - PS_P[p0]
                nc.sync.dma_start(gself[:], h_shard_p[p0][r0:r0 + 128, 0:HC])
                rhs_s = sb.tile([P, 130], f16, tag="rhss", bufs=2)
                nc.vector.tensor_scalar(
                    out=rhs_s[:, 0:C], in0=gself[:, 0:C],
                    scalar1=expv_s[:, 2 * w:2 * w + 1], scalar2=None,
                    op0=OP.mult)
                nc.scalar.activation(
                    rhs_s[:, C:HC], gself[:, C:HC], AF.Copy,
                    scale=expv_s[:, 2 * w + 1:2 * w + 2])
                nc.vector.tensor_copy(rhs_s[:, HC:HC + 2],
                                      expv_s[:, 2 * w:2 * w + 2])
                agg_t, ao = agg_slice(w)
                nc.tensor.matmul(agg_t[:, ao:ao + HC + 2], lhsT=ident[:],
                                 rhs=rhs_s[:], start=False, stop=True)
                recip = sb.tile([P, 2], f32, tag="recip", bufs=3)
                nc.vector.reciprocal(recip[:], agg_t[:, ao + HC:ao + HC + 2])
                oslice = out_acc[:, w * HC:(w + 1) * HC]
                for h in range(H):
                    if has_bias:
                        tmp = sb.tile([P, C], f32, tag="tmpb", bufs=2)
                        nc.vector.tensor_scalar(
                            out=tmp[:],
                            in0=agg_t[:, ao + C * h:ao + C * (h + 1)],
                            scalar1=recip[:, h:h + 1], scalar2=None,
                            op0=OP.mult)
                        nc.vector.tensor_tensor(
                            out=tmp[:], in0=tmp[:],
                            in1=bias_bc[:, C * h:C * (h + 1)], op=OP.add)
                        nc.vector.tensor_scalar(
                            out=oslice[:, C * h:C * (h + 1)], in0=tmp[:],
                            scalar1=0.0, scalar2=None, op0=OP.max)
                    else:
                        nc.vector.tensor_scalar(
                            out=oslice[:, C * h:C * (h + 1)],
                            in0=agg_t[:, ao + C * h:ao + C * (h + 1)],
                            scalar1=recip[:, h:h + 1], scalar2=0.0,
                            op0=OP.mult, op1=OP.max)
                agg_tiles.pop(w)
                sq_t = sb.tile([P, HC], f16, tag="sq", bufs=2)
                nc.vector.tensor_tensor(out=sq_t[:], in0=oslice, in1=oslice,
                                        op=OP.mult)
                nc.tensor.matmul(stats_ps[:, 0:HC], lhsT=ones16[:],
                                 rhs=oslice, start=False,
                                 stop=(nfin[0] == NB - 1))
                nc.tensor.matmul(stats_ps[:, HC:2 * HC], lhsT=ones16[:],
                                 rhs=sq_t[:], start=False,
                                 stop=(nfin[0] == NB - 1))
                nfin[0] += 1

            if _dbg:
                tb = sb.tile([P, ROWW], f32, tag="dbg", bufs=1)
                bsh = sb.tile([P, ROWW], f16, tag="dbgh", bufs=1)
                nc.sync.dma_start(bsh[:], h_shard_p[0][0:128, :])
                nc.vector.tensor_copy(tb[:], bsh[:])
                nc.sync.dma_start(dbg_hsh[:], tb[:])
                tb2 = sb.tile([P, ROWW], f32, tag="dbg2", bufs=1)
                bfu = sb.tile([P, ROWW], f16, tag="dbgh2", bufs=1)
                nc.sync.dma_start(bfu[:], h_full_p[0][QSH_P[0] * 1:
                                                      QSH_P[0] * 1 + 128, :])
                nc.vector.tensor_copy(tb2[:], bfu[:])
                nc.sync.dma_start(dbg_hfu[:], tb2[:])
            dbg_done = [False]

            # progress-ordered batches: by starting block, then stream
            border = sorted(
                [(q, b) for q in range(QS) for b in range(nbatch_q[q])],
                key=lambda qb: (int(blockof[qb[0]][min(qb[1] * BCH,
                                len(blockof[qb[0]]) - 1)]), qb[0]))
            for (q, b) in border:
                    prefetch(q, b + PREFETCH)
                    G = gtiles[(q, b)]
                    Gv = G[:].rearrange("p (k d) -> p k d", d=ROWW)
                    nch = min(BCH, int(SK_q[q]) - b * BCH)
                    gc0 = int(off_q[q]) + b * BCH         # first chunk col
                    esc_ps = psp.tile([P, 2 * BCH], f32, tag="escp", bufs=2)
                    eqT_bt = sb.tile([P, BCH * P], f16, tag="eqt", bufs=3)
                    nc.sync.dma_start(
                        eqT_bt[:, 0:nch * 128],
                        eqT_in[:, gc0 * 128:(gc0 + nch) * 128])
                    eqs = []
                    for k in range(nch):
                        ci = b * BCH + k                  # stream chunk idx
                        w = int(blockof[q][ci])
                        eq = sb.tile([P, P], f16, tag="eq", bufs=2 * BCH + 2)
                        nc.vector.tensor_scalar(
                            out=eq[:], in0=iota16[:],
                            scalar1=dl_sb[:, gc0 + k:gc0 + k + 1],
                            scalar2=None, op0=OP.is_equal)
                        eqs.append(eq)
                        nc.tensor.matmul(
                            esc_ps[:, 2 * k:2 * k + 2],
                            lhsT=eqT_bt[:, k * 128:(k + 1) * 128],
                            rhs=sc_acc[:, 4 * w + 2:4 * w + 4],
                            start=True, stop=True)
                    esc_sb = sb.tile([P, 2 * BCH], f32, tag="escs", bufs=3)
                    nc.vector.tensor_tensor(
                        out=esc_sb[:, 0:2 * nch],
                        in0=esc_ps[:, 0:2 * nch],
                        in1=Gv[:, 0:nch, 128:130], op=OP.add)
                    t02 = sb.tile([P, 2 * BCH], f32, tag="t02", bufs=3)
                    nc.vector.tensor_scalar(
                        out=t02[:, 0:2 * nch], in0=esc_sb[:, 0:2 * nch],
                        scalar1=NEG_SLOPE, scalar2=None, op0=OP.mult)
                    lr = sb.tile([P, 2 * BCH], f32, tag="lr", bufs=3)
                    nc.vector.tensor_tensor(
                        out=lr[:, 0:2 * nch], in0=t02[:, 0:2 * nch],
                        in1=esc_sb[:, 0:2 * nch], op=OP.max)
                    expv = sb.tile([P, 2 * BCH], f32, tag="expv", bufs=3)
                    nc.scalar.activation(expv[:, 0:2 * nch],
                                         lr[:, 0:2 * nch], AF.Exp)
                    rhs = sb.tile([P, BCH * 130], f16, tag="rhs", bufs=3)
                    nc.vector.tensor_copy(
                        rhs[:].rearrange(
                            "p (k d) -> p k d", d=130)[:, 0:nch, 128:130],
                        expv[:].rearrange(
                            "p (k d) -> p k d", d=2)[:, 0:nch, :])
                    for k in range(nch):
                        ci = b * BCH + k
                        w = int(blockof[q][ci])
                        nc.vector.tensor_scalar(
                            out=rhs[:, 130 * k:130 * k + C],
                            in0=G[:, ROWW * k:ROWW * k + C],
                            scalar1=expv[:, 2 * k:2 * k + 1], scalar2=None,
                            op0=OP.mult)
                        nc.scalar.activation(
                            rhs[:, 130 * k + C:130 * k + HC],
                            G[:, ROWW * k + C:ROWW * k + HC],
                            AF.Copy, scale=expv[:, 2 * k + 1:2 * k + 2])
                        agg_t, ao = agg_slice(w)
                        first = w not in started
                        started.add(w)
                        nc.tensor.matmul(
                            agg_t[:, ao:ao + HC + 2], lhsT=eqs[k][:],
                            rhs=rhs[:, 130 * k:130 * (k + 1)],
                            start=first, stop=False)
                        remaining[w] -= 1
                        if remaining[w] == 0:
                            finalize(w)
                    if _dbg and q == 0 and b == 0 and not dbg_done[0]:
                        dbg_done[0] = True
                        tg = sb.tile([P, BCH * ROWW], f32, tag="dbgg", bufs=1)
                        nc.vector.tensor_copy(tg[:], G[:])
                        nc.sync.dma_start(dbg_g[:], tg[:])
                        te = sb.tile([P, 2 * BCH], f32, tag="dbge", bufs=1)
                        nc.vector.tensor_copy(te[:], esc_sb[:])
                        nc.sync.dma_start(dbg_esc[:], te[:])
                        tr = sb.tile([P, BCH * 130], f32, tag="dbgr", bufs=1)
                        nc.vector.tensor_copy(tr[:], rhs[:])
                        nc.sync.dma_start(dbg_rhs[:], tr[:])

            if _dbg:
                for w in range(NB):
                    finp = sb.tile([P, HC], f32, tag="dbgp", bufs=3)
                    nc.vector.tensor_copy(
                        finp[:], out_acc[:, w * HC:(w + 1) * HC])
                    nc.sync.dma_start(
                        dbg_pre[w * 128:(w + 1) * 128, :], finp[:])

            # ---------------- BN epilogue ----------------
            st_sb = sb.tile([1, 2 * HC], f32, tag="st", bufs=1)
            nc.vector.tensor_copy(st_sb[:], stats_ps[:])
            st_loc = dram.tile([1, 2 * HC], f32)
            st_glob = dram.tile([1, 2 * HC], f32)
            nc.sync.dma_start(st_loc[:], st_sb[:])
            nc.gpsimd.collective_compute(
                "AllReduce", OP.add,
                replica_groups=[list(range(NCORES))],
                ins=[st_loc[:].opt()], outs=[st_glob[:].opt()])
            st_g = sb.tile([1, 2 * HC], f32, tag="stg", bufs=1)
            nc.sync.dma_start(st_g[:], st_glob[:])

            sc2 = sb.tile([1, 2 * HC], f32, tag="sc2", bufs=1)
            mrow = sb.tile([1, HC], f32, tag="mrow", bufs=1)
            nc.vector.tensor_scalar(out=mrow[:], in0=st_g[:, 0:HC],
                                    scalar1=1.0 / N, scalar2=None,
                                    op0=OP.mult)
            vrow = sb.tile([1, HC], f32, tag="vrow", bufs=1)
            nc.vector.tensor_scalar(out=vrow[:], in0=st_g[:, HC:2 * HC],
                                    scalar1=1.0 / N, scalar2=None,
                                    op0=OP.mult)
            m2 = sb.tile([1, HC], f32, tag="m2", bufs=1)
            nc.vector.tensor_tensor(out=m2[:], in0=mrow[:], in1=mrow[:],
                                    op=OP.mult)
            nc.vector.tensor_tensor(out=vrow[:], in0=vrow[:], in1=m2[:],
                                    op=OP.subtract)
            nc.vector.tensor_scalar(out=vrow[:], in0=vrow[:],
                                    scalar1=BN_EPS, scalar2=None, op0=OP.add)
            rinv = sb.tile([1, HC], f32, tag="rinv", bufs=1)
            nc.vector.reciprocal(rinv[:], vrow[:])
            rstd = sb.tile([1, HC], f32, tag="rstd", bufs=1)
            nc.scalar.activation(rstd[:], rinv[:], AF.Sqrt)
            nc.vector.tensor_tensor(out=sc2[:, 0:HC], in0=gbb_sb[:, 0:HC],
                                    in1=rstd[:], op=OP.mult)
            msc = sb.tile([1, HC], f32, tag="msc", bufs=1)
            nc.vector.tensor_tensor(out=msc[:], in0=mrow[:],
                                    in1=sc2[:, 0:HC], op=OP.mult)
            nc.vector.tensor_tensor(out=sc2[:, HC:2 * HC],
                                    in0=gbb_sb[:, HC:2 * HC],
                                    in1=msc[:], op=OP.subtract)
            bc_ps = psp.tile([P, 2 * HC], f32, tag="escp", bufs=2)
            nc.tensor.matmul(bc_ps[:], lhsT=ones_row[:], rhs=sc2[:],
                             start=True, stop=True)
            bc_sb = sb.tile([P, 2 * HC], f32, tag="bc", bufs=1)
            nc.vector.tensor_copy(bc_sb[:], bc_ps[:])

            for w in range(NB):
                fin = sb.tile([P, HC], f32, tag="fin", bufs=3)
                nc.vector.tensor_tensor(out=fin[:],
                                        in0=out_acc[:, w * HC:(w + 1) * HC],
                                        in1=bc_sb[:, 0:HC], op=OP.mult)
                nc.vector.tensor_tensor(out=fin[:], in0=fin[:],
                                        in1=bc_sb[:, HC:2 * HC], op=OP.add)
                nc.sync.dma_start(out_dram[w * 128:(w + 1) * 128, :], fin[:])

    lower_extended_insts(nc)
    _split_waits(nc, mybir)
    return nc


_CACHE = {}


def kernel(**inputs):
    x = inputs["x"]
    edge_index = inputs["edge_index"]
    W = inputs["W"]
    att_src = inputs["att_src"]
    att_dst = inputs["att_dst"]
    bias = inputs["bias"]
    gamma = inputs["gamma"]
    beta = inputs["beta"]

    per_core, meta = _host_prep(x, edge_index, W, att_src, att_dst,
                                bias, gamma, beta)
    has_bias = bool(np.any(np.asarray(bias) != 0))

    import os as _os
    key = ("prog2", tuple(meta["K"].reshape(-1).tolist()), has_bias,
           bool(_os.environ.get("KERNEL_DEBUG")))
    if key in _CACHE:
        nc = _CACHE[key]
    else:
        nc = _build_program(meta, has_bias)
        _CACHE[key] = nc

    from concourse.bass_utils import run_bass_kernel_spmd
    res = run_bass_kernel_spmd(nc, per_core, core_ids=list(range(NCORES)))

    out = np.zeros((N, HC), dtype=np.float32)
    for c in range(NCORES):
        shard = res.results[c]["out_shard"]          # [NSH, HC] rank-ordered
        order = meta["orders"][c]
        out[c * NSH_RAW + order] = shard[:NSH_RAW]
    return out


# revision 29
# speedup vs baseline: 1.4644x; 1.0333x over previous
"""GAT (2-head, 64-ch) + BatchNorm message passing on 8 Trainium2 cores.

Dst-node graph parallel: 12500 dst nodes/core (98 blocks x 128, in-degree
sorted so per-block edge counts are uniform across cores). Phase 0 computes
h_aug = x @ [W | W@att_src^T | W@att_dst^T] per shard into 512-byte table
rows [h(128f16) | a_src(2) | a_dst(2) | pad]; the table is AllGathered in 4
rank-quarter pieces so quartile-q gathers can start as soon as piece q
lands. Edges are routed to the dst core, bucketed per (dst block, src
quartile) padded to 128-edge chunks (uniform chunk grid across cores), and
gathered 8 chunks (1024 rows) per dma_gather on 4 SWDGE queues with
prefetch. Per chunk: a_dst expand via one PE matmul with a host-uploaded
transposed one-hot (eqT); esc/leaky-relu/exp batched per 8-chunk batch;
w-scaled values built per head on DVE/ACT; one PE matmul per chunk
accumulates numerator and denominator into the block PSUM. Block finalize
adds the self-loop (scores straight from phase-0 columns), normalizes,
applies ReLU, and accumulates BN stats; stats are AllReduced and the BN
affine applied in a final pass.
"""
import sys
sys.path.insert(0, "/opt/trn_rl_repo")
import numpy as np

N = 100_000
F = 128
H = 2
C = 64
HC = H * C
NEG_SLOPE = 0.2
BN_EPS = 1e-5
NCORES = 8
NSH_RAW = 12_500
NSH = 12_544            # 98 * 128
NB = NSH // 128         # 98
QS = 4
# block-aligned shard quarters (ranks); quartile tables are 8x these rows
QSH_P = [3200, 3200, 3072, 3072]
PS_P = [0, 3200, 6400, 9472, 12544]
P = 128
ROWW = 256              # table row width in f16 (512 B)
BCH = 8                 # chunks per gather batch (1024 rows)
BROWS = BCH * P
PADVAL = 200.0
PREFETCH = 3


def _host_prep(x, edge_index, W, att_src, att_dst, bias, gamma, beta):
    src = np.asarray(edge_index[0]).astype(np.int64)
    dst = np.asarray(edge_index[1]).astype(np.int64)
    x = np.asarray(x, dtype=np.float32)
    W = np.asarray(W, dtype=np.float32)
    att_src = np.asarray(att_src, dtype=np.float32)
    att_dst = np.asarray(att_dst, dtype=np.float32)

    W_aug = np.zeros((F, 132), dtype=np.float32)
    W_aug[:, 0:HC] = W
    W_aug[:, HC:HC + 2] = np.einsum(
        "fhc,hc->fh", W.reshape(F, H, C), att_src)
    W_aug[:, HC + 2:HC + 4] = np.einsum(
        "fhc,hc->fh", W.reshape(F, H, C), att_dst)

    gbb = np.zeros((1, 3 * HC), dtype=np.float32)
    gbb[0, 0:HC] = np.asarray(gamma, dtype=np.float32).reshape(-1)
    gbb[0, HC:2 * HC] = np.asarray(beta, dtype=np.float32).reshape(-1)
    gbb[0, 2 * HC:] = np.asarray(bias, dtype=np.float32).reshape(-1)

    # per-core in-degree rank (degree-sorted blocks)
    orders, ranks = [], []
    for c in range(NCORES):
        m = (dst // NSH_RAW) == c
        d_loc = dst[m] - c * NSH_RAW
        deg = np.bincount(d_loc, minlength=NSH_RAW)
        order = np.argsort(-deg, kind="stable")
        rank = np.empty(NSH_RAW, dtype=np.int64)
        rank[order] = np.arange(NSH_RAW)
        orders.append(order)
        ranks.append(rank)

    ps = np.asarray(PS_P, dtype=np.int64)
    qsh = np.asarray(QSH_P, dtype=np.int64)

    # per-core edge bucketing by (dst block, src quartile)
    core_ed = []
    cnts = np.zeros((NCORES, NB, QS), dtype=np.int64)
    for c in range(NCORES):
        m = (dst // NSH_RAW) == c
        s_c = src[m]
        r_d = ranks[c][dst[m] - c * NSH_RAW]
        w = r_d // 128
        j = r_d % 128
        cs = s_c // NSH_RAW
        r_s_local = np.empty(len(s_c), dtype=np.int64)
        for c2 in range(NCORES):
            mm = cs == c2
            r_s_local[mm] = ranks[c2][s_c[mm] - c2 * NSH_RAW]
        q = (np.searchsorted(ps, r_s_local, side="right") - 1).astype(np.int64)
        idx16 = cs * qsh[q] + (r_s_local - ps[q])
        core_ed.append((w, j, q, idx16))
        np.add.at(cnts[c], (w, q), 1)

    K = ((cnts.max(axis=0) + 127) // 128).astype(np.int64)   # [NB, QS]
    SK_q = K.sum(axis=0)                                     # chunks/stream
    TOTCH = int(SK_q.sum())
    nbatch_q = [int((SK_q[q] + BCH - 1) // BCH) for q in range(QS)]
    base_wq = np.zeros((NB, QS), dtype=np.int64)             # chunk base of (w,q)
    for q in range(QS):
        base_wq[1:, q] = np.cumsum(K[:-1, q])
    off_q = np.zeros(QS, dtype=np.int64)                     # stream col offset
    off_q[1:] = np.cumsum(SK_q[:-1])

    per_core = []
    for c in range(NCORES):
        w, j, q, idx16 = core_ed[c]
        ordv = np.lexsort((idx16, j, w + NB * q))
        wq, jq, qq, iq = w[ordv], j[ordv], q[ordv], idx16[ordv]

        idx_streams = [np.zeros(nbatch_q[s] * BROWS, dtype=np.int16)
                       for s in range(QS)]
        dl = np.full((TOTCH * 128,), PADVAL, dtype=np.float32)
        eqT = np.zeros((128, TOTCH * 128), dtype=np.float16)
        eqN = np.zeros((128, TOTCH * 128), dtype=np.float16)
        for s in range(QS):
            ms = qq == s
            ws, js, is_ = wq[ms], jq[ms], iq[ms]
            blo = np.searchsorted(ws, np.arange(NB))
            bhi = np.searchsorted(ws, np.arange(NB) + 1)
            for wv in range(NB):
                a, b = int(blo[wv]), int(bhi[wv])
                ne = b - a
                if ne == 0:
                    continue
                sbase = int(base_wq[wv, s]) * 128          # row in stream s
                idx_streams[s][sbase:sbase + ne] = is_[a:b].astype(np.int16)
                gch = (int(off_q[s]) + int(base_wq[wv, s])) * 128
                dl[gch:gch + ne] = js[a:b].astype(np.float32)
                eqT[js[a:b], gch + np.arange(ne)] = 1.0
                # eq rows: partition = edge slot within its chunk
                esl = (gch + np.arange(ne)) % 128
                ech = (gch + np.arange(ne)) // 128
                eqN[esl, ech * 128 + js[a:b]] = 1.0

        inp = {
            "xT": None,          # filled below
            "W_aug": W_aug,
            "gbb": gbb,
            "dl": np.ascontiguousarray(
                dl.reshape(TOTCH, 128).T).astype(np.float32),
            "eqT": eqT,
            "eqN": eqN,
        }
        for s in range(QS):
            lin = idx_streams[s]
            wrapped = lin.reshape(-1, 16).T                  # [16, nb*64]
            arr = np.zeros((P, nbatch_q[s] * (BROWS // 16)), dtype=np.int16)
            for grp in range(8):
                arr[grp * 16:(grp + 1) * 16, :] = wrapped
            inp[f"idx16_{s}"] = arr

        xs = np.zeros((NSH, F), dtype=np.float32)
        xs[:NSH_RAW] = x[c * NSH_RAW:(c + 1) * NSH_RAW][orders[c]]
        inp["xT"] = np.ascontiguousarray(xs.T).astype(np.float16)
        per_core.append(inp)

    meta = dict(K=K, SK_q=SK_q, TOTCH=TOTCH, nbatch_q=nbatch_q,
                base_wq=base_wq, off_q=off_q, orders=orders)
    return per_core, meta


def _split_waits(nc, mybir, keep=1):
    """Walrus accepts at most one sem-wait on DMA/CTRL pseudo instructions;
    hoist excess waits onto InstEventSemaphore."""
    for f in nc.m.functions:
        for bb in f.blocks:
            new = []
            for ins in bb.instructions:
                si = ins.sync_info
                if si is not None and si.on_wait and len(si.on_wait) > keep:
                    for jj, wcond in enumerate(list(si.on_wait)[:-keep]):
                        w = mybir.InstEventSemaphore(
                            name=f"{ins.name}-ws{jj}", ins=[], outs=[])
                        w.engine = ins.engine
                        w.sync_info = mybir.SyncInfo(
                            on_wait=[wcond], on_update=[])
                        new.append(w)
                    ins.sync_info = mybir.SyncInfo(
                        on_wait=list(si.on_wait)[-keep:],
                        on_update=list(si.on_update))
                new.append(ins)
            bb.instructions[:] = new


def _build_program(meta, has_bias):
    import concourse.bass as bass
    import concourse.mybir as mybir
    import concourse.tile as tile
    from concourse.masks import make_identity
    from concourse.library_config import mlp as mlp_lib
    from concourse.library_overlay import lower_extended_insts

    K = meta["K"]; SK_q = meta["SK_q"]; TOTCH = meta["TOTCH"]
    nbatch_q = meta["nbatch_q"]; base_wq = meta["base_wq"]
    off_q = meta["off_q"]
    f16 = mybir.dt.float16
    f32 = mybir.dt.float32
    i16 = mybir.dt.int16
    AF = mybir.ActivationFunctionType
    OP = mybir.AluOpType

    nc = bass.Bass(num_devices=NCORES, num_swdge_queues=QS)
    xT_in = nc.dram_tensor("xT", [F, NSH], f16, kind="ExternalInput")
    W_in = nc.dram_tensor("W_aug", [F, 132], f32, kind="ExternalInput")
    gbb_in = nc.dram_tensor("gbb", [1, 3 * HC], f32, kind="ExternalInput")
    dl_in = nc.dram_tensor("dl", [P, TOTCH], f32, kind="ExternalInput")
    eqT_in = nc.dram_tensor("eqT", [P, TOTCH * 128], f16,
                            kind="ExternalInput")
    eqN_in = nc.dram_tensor("eqN", [P, TOTCH * 128], f16,
                            kind="ExternalInput")
    idx_in = [nc.dram_tensor(f"idx16_{q}", [P, nbatch_q[q] * (BROWS // 16)],
                             i16, kind="ExternalInput") for q in range(QS)]
    out_dram = nc.dram_tensor("out_shard", [NSH, HC], f32,
                              kind="ExternalOutput")
    import os as _os
    _dbg = bool(_os.environ.get("KERNEL_DEBUG"))
    if _dbg:
        dbg_g = nc.dram_tensor("dbg_g", [P, BCH * ROWW], f32,
                               kind="ExternalOutput")
        dbg_esc = nc.dram_tensor("dbg_esc", [P, 2 * BCH], f32,
                                 kind="ExternalOutput")
        dbg_rhs = nc.dram_tensor("dbg_rhs", [P, BCH * 130], f32,
                                 kind="ExternalOutput")
        dbg_pre = nc.dram_tensor("dbg_pre", [NSH, HC], f32,
                                 kind="ExternalOutput")
        dbg_hsh = nc.dram_tensor("dbg_hsh", [P, ROWW], f32,
                                 kind="ExternalOutput")
        dbg_hfu = nc.dram_tensor("dbg_hfu", [P, ROWW], f32,
                                 kind="ExternalOutput")

    # chunk -> block map per stream
    blockof = [np.repeat(np.arange(NB), K[:, q]) for q in range(QS)]
    # first/last chunk of each block (global over the 4 streams' chunklists)
    remaining0 = K.sum(axis=1)

    with tile.TileContext(nc) as tc:
        with tc.tile_pool(name="cst", bufs=1) as cst, \
             tc.tile_pool(name="sb", bufs=2) as sb, \
             tc.tile_pool(name="ps", bufs=1, space="PSUM") as psp, \
             tc.tile_pool(name="dram", bufs=1, space="DRAM") as dram:

            ident = cst.tile([P, P], f16)
            make_identity(nc, ident[:])
            iota_i = cst.tile([P, P], mybir.dt.int32)
            nc.gpsimd.iota(iota_i[:], pattern=[[1, P]], channel_multiplier=0)
            iota16 = cst.tile([P, P], f16)
            nc.vector.tensor_copy(iota16[:], iota_i[:])
            ones16 = cst.tile([P, 1], f16)
            nc.vector.memset(ones16[:], 1.0)
            ones_row = cst.tile([1, P], f32)
            nc.vector.memset(ones_row[:], 1.0)
            W_f32 = cst.tile([F, 132], f32)
            nc.sync.dma_start(W_f32[:], W_in[:])
            W_sb = cst.tile([F, 132], f16)
            nc.vector.tensor_copy(W_sb[:], W_f32[:])
            gbb_sb = cst.tile([1, 3 * HC], f32)
            nc.sync.dma_start(gbb_sb[:], gbb_in[:])
            dl_sb = cst.tile([P, TOTCH], f32)
            nc.sync.dma_start(dl_sb[:], dl_in[:])
            idx_sb = []
            for q in range(QS):
                t = cst.tile([P, nbatch_q[q] * (BROWS // 16)], i16,
                             name=f"idxsb{q}")
                nc.sync.dma_start(t[:], idx_in[q][:])
                idx_sb.append(t)
            sc_acc = cst.tile([P, NB * 4], f16)
            out_acc = cst.tile([P, NB * HC], f16)

            nc.gpsimd.load_library(mlp_lib)

            # ---------------- phase 0: augmented h table ----------------
            h_shard_p = [dram.tile([QSH_P[p], ROWW], f16, name=f"hsh{p}")
                         for p in range(QS)]
            h_full_p = [dram.tile([NCORES * QSH_P[p], ROWW], f16,
                                  name=f"hfu{p}") for p in range(QS)]
            ag_after = {(PS_P[p + 1] - 1) // 128: p for p in range(QS)}

            nidx_regs = {}

            def reg_for(v):
                if v not in nidx_regs:
                    nidx_regs[v] = nc.gpsimd.to_reg(v)
                return nidx_regs[v]

            gtiles = {}

            def issue_gather(q, b):
                gt = sb.tile([P, BCH * ROWW], f16, tag=f"g{q}", bufs=4,
                             name=f"g{q}_{b}")
                nc.gpsimd.dma_gather(
                    out_ap=gt[:].rearrange("p (k d) -> p k d", d=ROWW),
                    in_ap=h_full_p[q][:, :],
                    idxs_ap=idx_sb[q][:, b * (BROWS // 16):
                                      (b + 1) * (BROWS // 16)],
                    num_idxs=BROWS,
                    num_idxs_reg=reg_for(BROWS),
                    elem_size=ROWW,
                    single_packet=False,
                    queue_num=q)
                gtiles[(q, b)] = gt

            next_issue = [0] * QS

            def prefetch(q, upto):
                while next_issue[q] <= min(upto, nbatch_q[q] - 1):
                    issue_gather(q, next_issue[q])
                    next_issue[q] += 1

            GRP = 14                       # blocks per phase-0 group
            for g in range(NB // GRP):
                xt = sb.tile([P, GRP * P], f16, tag="xt", bufs=2)
                nc.sync.dma_start(
                    xt[:], xT_in[:, g * GRP * 128:(g + 1) * GRP * 128])
                h_sb = sb.tile([P, GRP * ROWW], f16, tag="hsb", bufs=2)
                for v in range(GRP):
                    w = g * GRP + v
                    h_ps = psp.tile([P, 132], f32, tag="escp", bufs=2)
                    nc.tensor.matmul(h_ps[:], lhsT=xt[:, v * 128:
                                                      (v + 1) * 128],
                                     rhs=W_sb[:], start=True, stop=True)
                    nc.scalar.copy(
                        h_sb[:, v * ROWW:v * ROWW + 132], h_ps[:])
                    nc.vector.tensor_copy(sc_acc[:, 4 * w:4 * w + 4],
                                          h_ps[:, 128:132])
                # write group rows [1792g, 1792(g+1)) split by quarter piece
                r0, r1 = g * GRP * 128, (g + 1) * GRP * 128
                hv = h_sb[:].rearrange("p (v d) -> p v d", d=ROWW)
                for pc in range(QS):
                    a = max(r0, PS_P[pc])
                    bnd = min(r1, PS_P[pc + 1])
                    if a >= bnd:
                        continue
                    nc.sync.dma_start(
                        h_shard_p[pc][a - PS_P[pc]:bnd - PS_P[pc], :]
                        .rearrange("(v j) d -> j v d", j=P),
                        hv[:, (a - r0) // 128:(bnd - r0) // 128, :])
                for w in range(g * GRP, (g + 1) * GRP):
                    if w in ag_after:
                        p = ag_after[w]
                        nc.gpsimd.collective_compute(
                            "AllGather", OP.bypass,
                            replica_groups=[list(range(NCORES))],
                            ins=[h_shard_p[p][:].opt()],
                            outs=[h_full_p[p][:].opt()])
            for q in range(QS):
                prefetch(q, PREFETCH)

            # self-loop scores for all blocks at once
            sc_v = sc_acc[:].rearrange("p (w d) -> p w d", d=4)
            esc_s = cst.tile([P, NB * 2], f32)
            nc.vector.tensor_tensor(
                out=esc_s[:].rearrange("p (w d) -> p w d", d=2),
                in0=sc_v[:, :, 0:2], in1=sc_v[:, :, 2:4], op=OP.add)
            t02_s = cst.tile([P, NB * 2], f32)
            nc.vector.tensor_scalar(out=t02_s[:], in0=esc_s[:],
                                    scalar1=NEG_SLOPE, scalar2=None,
                                    op0=OP.mult)
            nc.vector.tensor_tensor(out=esc_s[:], in0=t02_s[:],
                                    in1=esc_s[:], op=OP.max)
            expv_s = cst.tile([P, NB * 2], f32)
            nc.scalar.activation(expv_s[:], esc_s[:], AF.Exp)

            if has_bias:
                bias_ps = psp.tile([P, HC], f32, tag="escp", bufs=2)
                nc.tensor.matmul(bias_ps[:], lhsT=ones_row[:],
                                 rhs=gbb_sb[:, 2 * HC:3 * HC],
                                 start=True, stop=True)
                bias_bc = cst.tile([P, HC], f32)
                nc.vector.tensor_copy(bias_bc[:], bias_ps[:])

            stats_ps = psp.tile([1, 2 * HC], f32, tag="stats", bufs=1)
            nc.vector.memset(stats_ps[:], 0.0)

            # ---------------- main loop (batch-major) ----------------
            agg_tiles = {}           # triple t -> psum tile [P, 3*130]
            triple_left = {}
            started = set()
            remaining = remaining0.copy()
            nfin = [0]

            def agg_slice(w):
                if w not in agg_tiles:
                    agg_tiles[w] = psp.tile([P, HC + 2], f32,
                                            tag="agg", bufs=5,
                                            name=f"agg{w}")
                return agg_tiles[w], 0

            def finalize(w):
                gself = sb.tile([P, HC], f16, tag="gself", bufs=3)
                p0 = next(p for p in range(QS)
                          if PS_P[p] <= w * 128 < PS_P[p + 1])
                r0 = w * 128 - PS_P[p0]
                nc.sync.dma_start(gself[:], h_shard_p[p0][r0:r0 + 128, 0:HC])
                rhs_s = sb.tile([P, 130], f16, tag="rhss", bufs=2)
                nc.vector.tensor_scalar(
                    out=rhs_s[:, 0:C], in0=gself[:, 0:C],
                    scalar1=expv_s[:, 2 * w:2 * w + 1], scalar2=None,
                    op0=OP.mult)
                nc.scalar.activation(
                    rhs_s[:, C:HC], gself[:, C:HC], AF.Copy,
                    scale=expv_s[:, 2 * w + 1:2 * w + 2])
                nc.vector.tensor_copy(rhs_s[:, HC:HC + 2],
                                      expv_s[:, 2 * w:2 * w + 2])
                agg_t, ao = agg_slice(w)
                nc.tensor.matmul(agg_t[:, ao:ao + HC + 2], lhsT=ident[:],
                                 rhs=rhs_s[:], start=False, stop=True)
                recip = sb.tile([P, 2], f32, tag="recip", bufs=3)
                nc.vector.reciprocal(recip[:], agg_t[:, ao + HC:ao + HC + 2])
                oslice = out_acc[:, w * HC:(w + 1) * HC]
                for h in range(H):
                    if has_bias:
                        tmp = sb.tile([P, C], f32, tag="tmpb", bufs=2)
                        nc.vector.tensor_scalar(
                            out=tmp[:],
                            in0=agg_t[:, ao + C * h:ao + C * (h + 1)],
                            scalar1=recip[:, h:h + 1], scalar2=None,
                            op0=OP.mult)
                        nc.vector.tensor_tensor(
                            out=tmp[:], in0=tmp[:],
                            in1=bias_bc[:, C * h:C * (h + 1)], op=OP.add)
                        nc.vector.tensor_scalar(
                            out=oslice[:, C * h:C * (h + 1)], in0=tmp[:],
                            scalar1=0.0, scalar2=None, op0=OP.max)
                    else:
                        nc.vector.tensor_scalar(
                            out=oslice[:, C * h:C * (h + 1)],
                            in0=agg_t[:, ao + C * h:ao + C * (h + 1)],
                            scalar1=recip[:, h:h + 1], scalar2=0.0,
                            op0=OP.mult, op1=OP.max)
                agg_tiles.pop(w)
                sq_t = sb.tile([P, HC], f16, tag="sq", bufs=2)
                nc.vector.tensor_tensor(out=sq_t[:], in0=oslice, in1=oslice,
                                        op=OP.mult)
                nc.tensor.matmul(stats_ps[:, 0:HC], lhsT=ones16[:],
                                 rhs=oslice, start=False,
                                 stop=(nfin[0] == NB - 1))
                nc.tensor.matmul(stats_ps[:, HC:2 * HC], lhsT=ones16[:],
                                 rhs=sq_t[:], start=False,
                                 stop=(nfin[0] == NB - 1))
                nfin[0] += 1

            if _dbg:
                tb = sb.tile([P, ROWW], f32, tag="dbg", bufs=1)
                bsh = sb.tile([P, ROWW], f16, tag="dbgh", bufs=1)
                nc.sync.dma_start(bsh[:], h_shard_p[0][0:128, :])
                nc.vector.tensor_copy(tb[:], bsh[:])
                nc.sync.dma_start(dbg_hsh[:], tb[:])
                tb2 = sb.tile([P, ROWW], f32, tag="dbg2", bufs=1)
                bfu = sb.tile([P, ROWW], f16, tag="dbgh2", bufs=1)
                nc.sync.dma_start(bfu[:], h_full_p[0][QSH_P[0] * 1:
                                                      QSH_P[0] * 1 + 128, :])
                nc.vector.tensor_copy(tb2[:], bfu[:])
                nc.sync.dma_start(dbg_hfu[:], tb2[:])
            dbg_done = [False]

            # progress-ordered batches: by starting block, then stream
            border = sorted(
                [(q, b) for q in range(QS) for b in range(nbatch_q[q])],
                key=lambda qb: (int(blockof[qb[0]][min(qb[1] * BCH,
                                len(blockof[qb[0]]) - 1)]), qb[0]))
            for (q, b) in border:
                    prefetch(q, b + PREFETCH)
                    G = gtiles[(q, b)]
                    Gv = G[:].rearrange("p (k d) -> p k d", d=ROWW)
                    nch = min(BCH, int(SK_q[q]) - b * BCH)
                    gc0 = int(off_q[q]) + b * BCH         # first chunk col
                    esc_ps = psp.tile([P, 2 * BCH], f32, tag="escp", bufs=2)
                    eqT_bt = sb.tile([P, BCH * P], f16, tag="eqt", bufs=3)
                    nc.sync.dma_start(
                        eqT_bt[:, 0:nch * 128],
                        eqT_in[:, gc0 * 128:(gc0 + nch) * 128])
                    eqN_bt = sb.tile([P, BCH * P], f16, tag="eqn", bufs=3)
                    nc.sync.dma_start(
                        eqN_bt[:, 0:nch * 128],
                        eqN_in[:, gc0 * 128:(gc0 + nch) * 128])
                    for k in range(nch):
                        ci = b * BCH + k                  # stream chunk idx
                        w = int(blockof[q][ci])
                        nc.tensor.matmul(
                            esc_ps[:, 2 * k:2 * k + 2],
                            lhsT=eqT_bt[:, k * 128:(k + 1) * 128],
                            rhs=sc_acc[:, 4 * w + 2:4 * w + 4],
                            start=True, stop=True)
                    esc_sb = sb.tile([P, 2 * BCH], f32, tag="escs", bufs=3)
                    nc.vector.tensor_tensor(
                        out=esc_sb[:, 0:2 * nch],
                        in0=esc_ps[:, 0:2 * nch],
                        in1=Gv[:, 0:nch, 128:130], op=OP.add)
                    t02 = sb.tile([P, 2 * BCH], f32, tag="t02", bufs=3)
                    nc.vector.tensor_scalar(
                        out=t02[:, 0:2 * nch], in0=esc_sb[:, 0:2 * nch],
                        scalar1=NEG_SLOPE, scalar2=None, op0=OP.mult)
                    lr = sb.tile([P, 2 * BCH], f32, tag="lr", bufs=3)
                    nc.vector.tensor_tensor(
                        out=lr[:, 0:2 * nch], in0=t02[:, 0:2 * nch],
                        in1=esc_sb[:, 0:2 * nch], op=OP.max)
                    expv = sb.tile([P, 2 * BCH], f32, tag="expv", bufs=3)
                    nc.scalar.activation(expv[:, 0:2 * nch],
                                         lr[:, 0:2 * nch], AF.Exp)
                    rhs = sb.tile([P, BCH * 130], f16, tag="rhs", bufs=3)
                    nc.vector.tensor_copy(
                        rhs[:].rearrange(
                            "p (k d) -> p k d", d=130)[:, 0:nch, 128:130],
                        expv[:].rearrange(
                            "p (k d) -> p k d", d=2)[:, 0:nch, :])
                    for k in range(nch):
                        ci = b * BCH + k
                        w = int(blockof[q][ci])
                        nc.vector.tensor_scalar(
                            out=rhs[:, 130 * k:130 * k + C],
                            in0=G[:, ROWW * k:ROWW * k + C],
                            scalar1=expv[:, 2 * k:2 * k + 1], scalar2=None,
                            op0=OP.mult)
                        nc.scalar.activation(
                            rhs[:, 130 * k + C:130 * k + HC],
                            G[:, ROWW * k + C:ROWW * k + HC],
                            AF.Copy, scale=expv[:, 2 * k + 1:2 * k + 2])
                        agg_t, ao = agg_slice(w)
                        first = w not in started
                        started.add(w)
                        nc.tensor.matmul(
                            agg_t[:, ao:ao + HC + 2],
                            lhsT=eqN_bt[:, k * 128:(k + 1) * 128],
                            rhs=rhs[:, 130 * k:130 * (k + 1)],
                            start=first, stop=False)
                        remaining[w] -= 1
                        if remaining[w] == 0:
                            finalize(w)
                    if _dbg and q == 0 and b == 0 and not dbg_done[0]:
                        dbg_done[0] = True
                        tg = sb.tile([P, BCH * ROWW], f32, tag="dbgg", bufs=1)
                        nc.vector.tensor_copy(tg[:], G[:])
                        nc.sync.dma_start(dbg_g[:], tg[:])
                        te = sb.tile([P, 2 * BCH], f32, tag="dbge", bufs=1)
                        nc.vector.tensor_copy(te[:], esc_sb[:])
                        nc.sync.dma_start(dbg_esc[:], te[:])
                        tr = sb.tile([P, BCH * 130], f32, tag="dbgr", bufs=1)
                        nc.vector.tensor_copy(tr[:], rhs[:])
                        nc.sync.dma_start(dbg_rhs[:], tr[:])

            if _dbg:
                for w in range(NB):
                    finp = sb.tile([P, HC], f32, tag="dbgp", bufs=3)
                    nc.vector.tensor_copy(
                        finp[:], out_acc[:, w * HC:(w + 1) * HC])
                    nc.sync.dma_start(
                        dbg_pre[w * 128:(w + 1) * 128, :], finp[:])

            # ---------------- BN epilogue ----------------
            st_sb = sb.tile([1, 2 * HC], f32, tag="st", bufs=1)
            nc.vector.tensor_copy(st_sb[:], stats_ps[:])
            st_loc = dram.tile([1, 2 * HC], f32)
            st_glob = dram.tile([1, 2 * HC], f32)
            nc.sync.dma_start(st_loc[:], st_sb[:])
            nc.gpsimd.collective_compute(
                "AllReduce", OP.add,
                replica_groups=[list(range(NCORES))],
                ins=[st_loc[:].opt()], outs=[st_glob[:].opt()])
            st_g = sb.tile([1, 2 * HC], f32, tag="stg", bufs=1)
            nc.sync.dma_start(st_g[:], st_glob[:])

            sc2 = sb.tile([1, 2 * HC], f32, tag="sc2", bufs=1)
            mrow = sb.tile([1, HC], f32, tag="mrow", bufs=1)
            nc.vector.tensor_scalar(out=mrow[:], in0=st_g[:, 0:HC],
                                    scalar1=1.0 / N, scalar2=None,
                                    op0=OP.mult)
            vrow = sb.tile([1, HC], f32, tag="vrow", bufs=1)
            nc.vector.tensor_scalar(out=vrow[:], in0=st_g[:, HC:2 * HC],
                                    scalar1=1.0 / N, scalar2=None,
                                    op0=OP.mult)
            m2 = sb.tile([1, HC], f32, tag="m2", bufs=1)
            nc.vector.tensor_tensor(out=m2[:], in0=mrow[:], in1=mrow[:],
                                    op=OP.mult)
            nc.vector.tensor_tensor(out=vrow[:], in0=vrow[:], in1=m2[:],
                                    op=OP.subtract)
            nc.vector.tensor_scalar(out=vrow[:], in0=vrow[:],
                                    scalar1=BN_EPS, scalar2=None, op0=OP.add)
            rinv = sb.tile([1, HC], f32, tag="rinv", bufs=1)
            nc.vector.reciprocal(rinv[:], vrow[:])
            rstd = sb.tile([1, HC], f32, tag="rstd", bufs=1)
            nc.scalar.activation(rstd[:], rinv[:], AF.Sqrt)
            nc.vector.tensor_tensor(out=sc2[:, 0:HC], in0=gbb_sb[:, 0:HC],
                                    in1=rstd[:], op=OP.mult)
            msc = sb.tile([1, HC], f32, tag="msc", bufs=1)
            nc.vector.tensor_tensor(out=msc[:], in0=mrow[:],
                                    in1=sc2[:, 0:HC], op=OP.mult)
            nc.vector.tensor_tensor(out=sc2[:, HC:2 * HC],
                                    in0=gbb_sb[:, HC:2 * HC],
                                    in1=msc[:], op=OP.subtract)
            bc_ps = psp.tile([P, 2 * HC], f32, tag="escp", bufs=2)
            nc.tensor.matmul(bc_ps[:], lhsT=ones_row[:], rhs=sc2[:],
                             start=True, stop=True)
            bc_sb = sb.tile([P, 2 * HC], f32, tag="bc", bufs=1)
            nc.vector.tensor_copy(bc_sb[:], bc_ps[:])

            FGRP = 14
            for g in range(NB // FGRP):
                fin = sb.tile([P, FGRP * HC], f32, tag="fin", bufs=2)
                for v in range(FGRP):
                    w = g * FGRP + v
                    nc.vector.tensor_tensor(
                        out=fin[:, v * HC:(v + 1) * HC],
                        in0=out_acc[:, w * HC:(w + 1) * HC],
                        in1=bc_sb[:, 0:HC], op=OP.mult)
                    nc.vector.tensor_tensor(
                        out=fin[:, v * HC:(v + 1) * HC],
                        in0=fin[:, v * HC:(v + 1) * HC],
                        in1=bc_sb[:, HC:2 * HC], op=OP.add)
                nc.sync.dma_start(
                    out_dram[g * FGRP * 128:(g + 1) * FGRP * 128, :]
                    .rearrange("(v j) d -> j v d", j=P),
                    fin[:].rearrange("p (v d) -> p v d", d=HC))

    lower_extended_insts(nc)
    _split_waits(nc, mybir)
    return nc


_CACHE = {}


def kernel(**inputs):
    x = inputs["x"]
    edge_index = inputs["edge_index"]
    W = inputs["W"]
    att_src = inputs["att_src"]
    att_dst = inputs["att_dst"]
    bias = inputs["bias"]
    gamma = inputs["gamma"]
    beta = inputs["beta"]

    per_core, meta = _host_prep(x, edge_index, W, att_src, att_dst,
                                bias, gamma, beta)
    has_bias = bool(np.any(np.asarray(bias) != 0))

    import os as _os
    key = ("prog2", tuple(meta["K"].reshape(-1).tolist()), has_bias,
           bool(_os.environ.get("KERNEL_DEBUG")))
    if key in _CACHE:
        nc = _CACHE[key]
    else:
        nc = _build_program(meta, has_bias)
        _CACHE[key] = nc

    from concourse.bass_utils import run_bass_kernel_spmd
    res = run_bass_kernel_spmd(nc, per_core, core_ids=list(range(NCORES)))

    out = np.zeros((N, HC), dtype=np.float32)
    for c in range(NCORES):
        shard = res.results[c]["out_shard"]          # [NSH, HC] rank-ordered
        order = meta["orders"][c]
        out[c * NSH_RAW + order] = shard[:NSH_RAW]
    return out


# revision 31
# speedup vs baseline: 1.5620x; 1.0666x over previous
"""GAT (2-head, 64-ch) + BatchNorm message passing on 8 Trainium2 cores.

Dst-node graph parallel: 12500 dst nodes/core (98 blocks x 128, in-degree
sorted so per-block edge counts are uniform across cores). Phase 0 computes
h_aug = x @ [W | W@att_src^T | W@att_dst^T] per shard into 512-byte table
rows [h(128f16) | a_src(2) | a_dst(2) | pad]; the table is AllGathered in 4
rank-quarter pieces so quartile-q gathers can start as soon as piece q
lands. Edges are routed to the dst core, bucketed per (dst block, src
quartile) padded to 128-edge chunks (uniform chunk grid across cores), and
gathered 8 chunks (1024 rows) per dma_gather on 4 SWDGE queues with
prefetch. Per chunk: a_dst expand via one PE matmul with a host-uploaded
transposed one-hot (eqT); esc/leaky-relu/exp batched per 8-chunk batch;
w-scaled values built per head on DVE/ACT; one PE matmul per chunk
accumulates numerator and denominator into the block PSUM. Block finalize
adds the self-loop (scores straight from phase-0 columns), normalizes,
applies ReLU, and accumulates BN stats; stats are AllReduced and the BN
affine applied in a final pass.
"""
import sys
sys.path.insert(0, "/opt/trn_rl_repo")
import numpy as np

N = 100_000
F = 128
H = 2
C = 64
HC = H * C
NEG_SLOPE = 0.2
BN_EPS = 1e-5
NCORES = 8
NSH_RAW = 12_500
NSH = 12_544            # 98 * 128
NB = NSH // 128         # 98
QS = 4
# block-aligned shard quarters (ranks); quartile tables are 8x these rows
QSH_P = [3200, 3200, 3072, 3072]
PS_P = [0, 3200, 6400, 9472, 12544]
P = 128
ROWW = 128              # table row width in f16 (256 B)
BCH = 16                # chunks per gather batch (2048 rows)
BROWS = BCH * P
PADVAL = 200.0
PREFETCH = 3


def _host_prep(x, edge_index, W, att_src, att_dst, bias, gamma, beta):
    src = np.asarray(edge_index[0]).astype(np.int64)
    dst = np.asarray(edge_index[1]).astype(np.int64)
    x = np.asarray(x, dtype=np.float32)
    W = np.asarray(W, dtype=np.float32)
    att_src = np.asarray(att_src, dtype=np.float32)
    att_dst = np.asarray(att_dst, dtype=np.float32)

    W_aug = np.zeros((F, 132), dtype=np.float32)
    W_aug[:, 0:HC] = W
    W_aug[:, HC:HC + 2] = np.einsum(
        "fhc,hc->fh", W.reshape(F, H, C), att_src)
    W_aug[:, HC + 2:HC + 4] = np.einsum(
        "fhc,hc->fh", W.reshape(F, H, C), att_dst)
    asrc_all = x @ W_aug[:, HC:HC + 2]                   # [N, 2] f32

    gbb = np.zeros((1, 3 * HC), dtype=np.float32)
    gbb[0, 0:HC] = np.asarray(gamma, dtype=np.float32).reshape(-1)
    gbb[0, HC:2 * HC] = np.asarray(beta, dtype=np.float32).reshape(-1)
    gbb[0, 2 * HC:] = np.asarray(bias, dtype=np.float32).reshape(-1)

    # per-core in-degree rank (degree-sorted blocks)
    orders, ranks = [], []
    for c in range(NCORES):
        m = (dst // NSH_RAW) == c
        d_loc = dst[m] - c * NSH_RAW
        deg = np.bincount(d_loc, minlength=NSH_RAW)
        order = np.argsort(-deg, kind="stable")
        rank = np.empty(NSH_RAW, dtype=np.int64)
        rank[order] = np.arange(NSH_RAW)
        orders.append(order)
        ranks.append(rank)

    ps = np.asarray(PS_P, dtype=np.int64)
    qsh = np.asarray(QSH_P, dtype=np.int64)

    # per-core edge bucketing by (dst block, src quartile)
    core_ed = []
    cnts = np.zeros((NCORES, NB, QS), dtype=np.int64)
    for c in range(NCORES):
        m = (dst // NSH_RAW) == c
        s_c = src[m]
        r_d = ranks[c][dst[m] - c * NSH_RAW]
        w = r_d // 128
        j = r_d % 128
        cs = s_c // NSH_RAW
        r_s_local = np.empty(len(s_c), dtype=np.int64)
        for c2 in range(NCORES):
            mm = cs == c2
            r_s_local[mm] = ranks[c2][s_c[mm] - c2 * NSH_RAW]
        q = (np.searchsorted(ps, r_s_local, side="right") - 1).astype(np.int64)
        idx16 = cs * qsh[q] + (r_s_local - ps[q])
        core_ed.append((w, j, q, idx16, s_c))
        np.add.at(cnts[c], (w, q), 1)

    K = ((cnts.max(axis=0) + 127) // 128).astype(np.int64)   # [NB, QS]
    SK_q = K.sum(axis=0)                                     # chunks/stream
    TOTCH = int(SK_q.sum())
    nbatch_q = [int((SK_q[q] + BCH - 1) // BCH) for q in range(QS)]
    base_wq = np.zeros((NB, QS), dtype=np.int64)             # chunk base of (w,q)
    for q in range(QS):
        base_wq[1:, q] = np.cumsum(K[:-1, q])
    off_q = np.zeros(QS, dtype=np.int64)                     # stream col offset
    off_q[1:] = np.cumsum(SK_q[:-1])

    per_core = []
    core_src = []
    for c in range(NCORES):
        core_src.append(None)
    for c in range(NCORES):
        w, j, q, idx16, s_c = core_ed[c]
        ordv = np.lexsort((idx16, j, w + NB * q))
        wq, jq, qq, iq = w[ordv], j[ordv], q[ordv], idx16[ordv]
        core_src[c] = s_c[ordv]

        import ml_dtypes
        f8 = ml_dtypes.float8_e4m3
        idx_streams = [np.zeros(nbatch_q[s] * BROWS, dtype=np.int16)
                       for s in range(QS)]
        eqT = np.zeros((128, TOTCH * 128), dtype=f8)
        eqN = np.zeros((128, TOTCH * 128), dtype=f8)
        asrcS = np.zeros((TOTCH * 128, 2), dtype=np.float16)
        src_glob = src  # global src ids (closure)
        sg = core_src[c]
        for s in range(QS):
            ms = qq == s
            ws, js, is_, gsrc = wq[ms], jq[ms], iq[ms], sg[ms]
            blo = np.searchsorted(ws, np.arange(NB))
            bhi = np.searchsorted(ws, np.arange(NB) + 1)
            for wv in range(NB):
                a, b = int(blo[wv]), int(bhi[wv])
                ne = b - a
                if ne == 0:
                    continue
                sbase = int(base_wq[wv, s]) * 128          # row in stream s
                idx_streams[s][sbase:sbase + ne] = is_[a:b].astype(np.int16)
                gch = (int(off_q[s]) + int(base_wq[wv, s])) * 128
                eqT[js[a:b], gch + np.arange(ne)] = 1.0
                # eq rows: partition = edge slot within its chunk
                esl = (gch + np.arange(ne)) % 128
                ech = (gch + np.arange(ne)) // 128
                eqN[esl, ech * 128 + js[a:b]] = 1.0
                asrcS[gch:gch + ne] = asrc_all[gsrc[a:b]].astype(np.float16)

        # [128 slot, TOTCH*2] layout: col ch*2+h
        asrc_t = np.ascontiguousarray(
            asrcS.reshape(TOTCH, 128, 2).transpose(1, 0, 2).reshape(
                128, TOTCH * 2))
        inp = {
            "xT": None,          # filled below
            "W_aug": W_aug,
            "gbb": gbb,
            "asrcS": asrc_t,
            "eqT": eqT,
            "eqN": eqN,
        }
        for s in range(QS):
            lin = idx_streams[s]
            wrapped = lin.reshape(-1, 16).T                  # [16, nb*64]
            arr = np.zeros((P, nbatch_q[s] * (BROWS // 16)), dtype=np.int16)
            for grp in range(8):
                arr[grp * 16:(grp + 1) * 16, :] = wrapped
            inp[f"idx16_{s}"] = arr

        xs = np.zeros((NSH, F), dtype=np.float32)
        xs[:NSH_RAW] = x[c * NSH_RAW:(c + 1) * NSH_RAW][orders[c]]
        inp["xT"] = np.ascontiguousarray(xs.T).astype(np.float16)
        per_core.append(inp)

    meta = dict(K=K, SK_q=SK_q, TOTCH=TOTCH, nbatch_q=nbatch_q,
                base_wq=base_wq, off_q=off_q, orders=orders)
    return per_core, meta


def _split_waits(nc, mybir, keep=1):
    """Walrus accepts at most one sem-wait on DMA/CTRL pseudo instructions;
    hoist excess waits onto InstEventSemaphore."""
    for f in nc.m.functions:
        for bb in f.blocks:
            new = []
            for ins in bb.instructions:
                si = ins.sync_info
                if si is not None and si.on_wait and len(si.on_wait) > keep:
                    for jj, wcond in enumerate(list(si.on_wait)[:-keep]):
                        w = mybir.InstEventSemaphore(
                            name=f"{ins.name}-ws{jj}", ins=[], outs=[])
                        w.engine = ins.engine
                        w.sync_info = mybir.SyncInfo(
                            on_wait=[wcond], on_update=[])
                        new.append(w)
                    ins.sync_info = mybir.SyncInfo(
                        on_wait=list(si.on_wait)[-keep:],
                        on_update=list(si.on_update))
                new.append(ins)
            bb.instructions[:] = new


def _build_program(meta, has_bias):
    import concourse.bass as bass
    import concourse.mybir as mybir
    import concourse.tile as tile
    from concourse.masks import make_identity
    from concourse.library_config import mlp as mlp_lib
    from concourse.library_overlay import lower_extended_insts

    K = meta["K"]; SK_q = meta["SK_q"]; TOTCH = meta["TOTCH"]
    nbatch_q = meta["nbatch_q"]; base_wq = meta["base_wq"]
    off_q = meta["off_q"]
    f16 = mybir.dt.float16
    f32 = mybir.dt.float32
    f8 = mybir.dt.float8e4
    i16 = mybir.dt.int16
    AF = mybir.ActivationFunctionType
    OP = mybir.AluOpType

    nc = bass.Bass(num_devices=NCORES, num_swdge_queues=QS)
    xT_in = nc.dram_tensor("xT", [F, NSH], f16, kind="ExternalInput")
    W_in = nc.dram_tensor("W_aug", [F, 132], f32, kind="ExternalInput")
    gbb_in = nc.dram_tensor("gbb", [1, 3 * HC], f32, kind="ExternalInput")
    asrc_in = nc.dram_tensor("asrcS", [P, TOTCH * 2], f16,
                             kind="ExternalInput")
    eqT_in = nc.dram_tensor("eqT", [P, TOTCH * 128], f8,
                            kind="ExternalInput")
    eqN_in = nc.dram_tensor("eqN", [P, TOTCH * 128], f8,
                            kind="ExternalInput")
    idx_in = [nc.dram_tensor(f"idx16_{q}", [P, nbatch_q[q] * (BROWS // 16)],
                             i16, kind="ExternalInput") for q in range(QS)]
    out_dram = nc.dram_tensor("out_shard", [NSH, HC], f32,
                              kind="ExternalOutput")
    import os as _os
    _dbg = bool(_os.environ.get("KERNEL_DEBUG"))
    if _dbg:
        dbg_g = nc.dram_tensor("dbg_g", [P, BCH * ROWW], f32,
                               kind="ExternalOutput")
        dbg_esc = nc.dram_tensor("dbg_esc", [P, 2 * BCH], f32,
                                 kind="ExternalOutput")
        dbg_rhs = nc.dram_tensor("dbg_rhs", [P, BCH * 130], f32,
                                 kind="ExternalOutput")
        dbg_pre = nc.dram_tensor("dbg_pre", [NSH, HC], f32,
                                 kind="ExternalOutput")
        dbg_hsh = nc.dram_tensor("dbg_hsh", [P, ROWW], f32,
                                 kind="ExternalOutput")
        dbg_hfu = nc.dram_tensor("dbg_hfu", [P, ROWW], f32,
                                 kind="ExternalOutput")

    # chunk -> block map per stream
    blockof = [np.repeat(np.arange(NB), K[:, q]) for q in range(QS)]
    # first/last chunk of each block (global over the 4 streams' chunklists)
    remaining0 = K.sum(axis=1)

    with tile.TileContext(nc) as tc:
        with tc.tile_pool(name="cst", bufs=1) as cst, \
             tc.tile_pool(name="sb", bufs=2) as sb, \
             tc.tile_pool(name="ps", bufs=1, space="PSUM") as psp, \
             tc.tile_pool(name="dram", bufs=1, space="DRAM") as dram:

            ident = cst.tile([P, P], f16)
            make_identity(nc, ident[:])
            iota_i = cst.tile([P, P], mybir.dt.int32)
            nc.gpsimd.iota(iota_i[:], pattern=[[1, P]], channel_multiplier=0)
            iota16 = cst.tile([P, P], f16)
            nc.vector.tensor_copy(iota16[:], iota_i[:])
            ones16 = cst.tile([P, 1], f16)
            nc.vector.memset(ones16[:], 1.0)
            ones_row = cst.tile([1, P], f32)
            nc.vector.memset(ones_row[:], 1.0)
            W_f32 = cst.tile([F, 132], f32)
            nc.sync.dma_start(W_f32[:], W_in[:])
            W_sb = cst.tile([F, 132], f16)
            nc.vector.tensor_copy(W_sb[:], W_f32[:])
            gbb_sb = cst.tile([1, 3 * HC], f32)
            nc.sync.dma_start(gbb_sb[:], gbb_in[:])
            asrc_sb = cst.tile([P, TOTCH * 2], f16)
            nc.sync.dma_start(asrc_sb[:], asrc_in[:])
            idx_sb = []
            for q in range(QS):
                t = cst.tile([P, nbatch_q[q] * (BROWS // 16)], i16,
                             name=f"idxsb{q}")
                nc.sync.dma_start(t[:], idx_in[q][:])
                idx_sb.append(t)
            sc_acc = cst.tile([P, NB * 4], f16)
            out_acc = cst.tile([P, NB * HC], f16)

            nc.gpsimd.load_library(mlp_lib)

            # ---------------- phase 0: augmented h table ----------------
            h_shard_p = [dram.tile([QSH_P[p], ROWW], f16, name=f"hsh{p}")
                         for p in range(QS)]
            h_full_p = [dram.tile([NCORES * QSH_P[p], ROWW], f16,
                                  name=f"hfu{p}") for p in range(QS)]
            ag_after = {(PS_P[p + 1] - 1) // 128: p for p in range(QS)}

            nidx_regs = {}

            def reg_for(v):
                if v not in nidx_regs:
                    nidx_regs[v] = nc.gpsimd.to_reg(v)
                return nidx_regs[v]

            gtiles = {}

            def issue_gather(q, b):
                gt = sb.tile([P, BCH * ROWW], f16, tag=f"g{q}", bufs=4,
                             name=f"g{q}_{b}")
                nc.gpsimd.dma_gather(
                    out_ap=gt[:].rearrange("p (k d) -> p k d", d=ROWW),
                    in_ap=h_full_p[q][:, :],
                    idxs_ap=idx_sb[q][:, b * (BROWS // 16):
                                      (b + 1) * (BROWS // 16)],
                    num_idxs=BROWS,
                    num_idxs_reg=reg_for(BROWS),
                    elem_size=ROWW,
                    single_packet=False,
                    queue_num=q)
                gtiles[(q, b)] = gt

            next_issue = [0] * QS

            def prefetch(q, upto):
                while next_issue[q] <= min(upto, nbatch_q[q] - 1):
                    issue_gather(q, next_issue[q])
                    next_issue[q] += 1

            GRP = 14                       # blocks per phase-0 group
            for g in range(NB // GRP):
                xt = sb.tile([P, GRP * P], f16, tag="xt", bufs=2)
                nc.sync.dma_start(
                    xt[:], xT_in[:, g * GRP * 128:(g + 1) * GRP * 128])
                h_sb = sb.tile([P, GRP * ROWW], f16, tag="hsb", bufs=2)
                for v in range(GRP):
                    w = g * GRP + v
                    h_ps = psp.tile([P, 132], f32, tag="escp", bufs=2)
                    nc.tensor.matmul(h_ps[:], lhsT=xt[:, v * 128:
                                                      (v + 1) * 128],
                                     rhs=W_sb[:], start=True, stop=True)
                    nc.scalar.copy(
                        h_sb[:, v * ROWW:v * ROWW + HC], h_ps[:, 0:HC])
                    nc.vector.tensor_copy(sc_acc[:, 4 * w:4 * w + 4],
                                          h_ps[:, 128:132])
                # write group rows [1792g, 1792(g+1)) split by quarter piece
                r0, r1 = g * GRP * 128, (g + 1) * GRP * 128
                hv = h_sb[:].rearrange("p (v d) -> p v d", d=ROWW)
                for pc in range(QS):
                    a = max(r0, PS_P[pc])
                    bnd = min(r1, PS_P[pc + 1])
                    if a >= bnd:
                        continue
                    nc.sync.dma_start(
                        h_shard_p[pc][a - PS_P[pc]:bnd - PS_P[pc], :]
                        .rearrange("(v j) d -> j v d", j=P),
                        hv[:, (a - r0) // 128:(bnd - r0) // 128, :])
                for w in range(g * GRP, (g + 1) * GRP):
                    if w in ag_after:
                        p = ag_after[w]
                        nc.gpsimd.collective_compute(
                            "AllGather", OP.bypass,
                            replica_groups=[list(range(NCORES))],
                            ins=[h_shard_p[p][:].opt()],
                            outs=[h_full_p[p][:].opt()])
            for q in range(QS):
                prefetch(q, PREFETCH)

            # self-loop scores for all blocks at once
            sc_v = sc_acc[:].rearrange("p (w d) -> p w d", d=4)
            esc_s = cst.tile([P, NB * 2], f32)
            nc.vector.tensor_tensor(
                out=esc_s[:].rearrange("p (w d) -> p w d", d=2),
                in0=sc_v[:, :, 0:2], in1=sc_v[:, :, 2:4], op=OP.add)
            t02_s = cst.tile([P, NB * 2], f32)
            nc.vector.tensor_scalar(out=t02_s[:], in0=esc_s[:],
                                    scalar1=NEG_SLOPE, scalar2=None,
                                    op0=OP.mult)
            nc.vector.tensor_tensor(out=esc_s[:], in0=t02_s[:],
                                    in1=esc_s[:], op=OP.max)
            expv_s = cst.tile([P, NB * 2], f32)
            nc.scalar.activation(expv_s[:], esc_s[:], AF.Exp)

            if has_bias:
                bias_ps = psp.tile([P, HC], f32, tag="escp", bufs=2)
                nc.tensor.matmul(bias_ps[:], lhsT=ones_row[:],
                                 rhs=gbb_sb[:, 2 * HC:3 * HC],
                                 start=True, stop=True)
                bias_bc = cst.tile([P, HC], f32)
                nc.vector.tensor_copy(bias_bc[:], bias_ps[:])

            stats_ps = psp.tile([1, 2 * HC], f32, tag="stats", bufs=1)
            nc.vector.memset(stats_ps[:], 0.0)

            # ---------------- main loop (batch-major) ----------------
            agg_tiles = {}           # triple t -> psum tile [P, 3*130]
            triple_left = {}
            started = set()
            remaining = remaining0.copy()
            nfin = [0]

            def agg_slice(w):
                if w not in agg_tiles:
                    agg_tiles[w] = psp.tile([P, HC + 2], f32,
                                            tag="agg", bufs=5,
                                            name=f"agg{w}")
                return agg_tiles[w], 0

            def finalize(w):
                gself = sb.tile([P, HC], f16, tag="gself", bufs=3)
                p0 = next(p for p in range(QS)
                          if PS_P[p] <= w * 128 < PS_P[p + 1])
                r0 = w * 128 - PS_P[p0]
                nc.sync.dma_start(gself[:], h_shard_p[p0][r0:r0 + 128, :])
                rhs_s = sb.tile([P, 130], f16, tag="rhss", bufs=2)
                nc.vector.tensor_scalar(
                    out=rhs_s[:, 0:C], in0=gself[:, 0:C],
                    scalar1=expv_s[:, 2 * w:2 * w + 1], scalar2=None,
                    op0=OP.mult)
                nc.scalar.activation(
                    rhs_s[:, C:HC], gself[:, C:HC], AF.Copy,
                    scale=expv_s[:, 2 * w + 1:2 * w + 2])
                nc.vector.tensor_copy(rhs_s[:, HC:HC + 2],
                                      expv_s[:, 2 * w:2 * w + 2])
                agg_t, ao = agg_slice(w)
                nc.tensor.matmul(agg_t[:, ao:ao + HC + 2], lhsT=ident[:],
                                 rhs=rhs_s[:], start=False, stop=True)
                recip = sb.tile([P, 2], f32, tag="recip", bufs=3)
                nc.vector.reciprocal(recip[:], agg_t[:, ao + HC:ao + HC + 2])
                oslice = out_acc[:, w * HC:(w + 1) * HC]
                for h in range(H):
                    if has_bias:
                        tmp = sb.tile([P, C], f32, tag="tmpb", bufs=2)
                        nc.vector.tensor_scalar(
                            out=tmp[:],
                            in0=agg_t[:, ao + C * h:ao + C * (h + 1)],
                            scalar1=recip[:, h:h + 1], scalar2=None,
                            op0=OP.mult)
                        nc.vector.tensor_tensor(
                            out=tmp[:], in0=tmp[:],
                            in1=bias_bc[:, C * h:C * (h + 1)], op=OP.add)
                        nc.vector.tensor_scalar(
                            out=oslice[:, C * h:C * (h + 1)], in0=tmp[:],
                            scalar1=0.0, scalar2=None, op0=OP.max)
                    else:
                        nc.vector.tensor_scalar(
                            out=oslice[:, C * h:C * (h + 1)],
                            in0=agg_t[:, ao + C * h:ao + C * (h + 1)],
                            scalar1=recip[:, h:h + 1], scalar2=0.0,
                            op0=OP.mult, op1=OP.max)
                agg_tiles.pop(w)
                sq_t = sb.tile([P, HC], f16, tag="sq", bufs=2)
                nc.vector.tensor_tensor(out=sq_t[:], in0=oslice, in1=oslice,
                                        op=OP.mult)
                nc.tensor.matmul(stats_ps[:, 0:HC], lhsT=ones16[:],
                                 rhs=oslice, start=False,
                                 stop=(nfin[0] == NB - 1))
                nc.tensor.matmul(stats_ps[:, HC:2 * HC], lhsT=ones16[:],
                                 rhs=sq_t[:], start=False,
                                 stop=(nfin[0] == NB - 1))
                nfin[0] += 1

            if _dbg:
                tb = sb.tile([P, ROWW], f32, tag="dbg", bufs=1)
                bsh = sb.tile([P, ROWW], f16, tag="dbgh", bufs=1)
                nc.sync.dma_start(bsh[:], h_shard_p[0][0:128, :])
                nc.vector.tensor_copy(tb[:], bsh[:])
                nc.sync.dma_start(dbg_hsh[:], tb[:])
                tb2 = sb.tile([P, ROWW], f32, tag="dbg2", bufs=1)
                bfu = sb.tile([P, ROWW], f16, tag="dbgh2", bufs=1)
                nc.sync.dma_start(bfu[:], h_full_p[0][QSH_P[0] * 1:
                                                      QSH_P[0] * 1 + 128, :])
                nc.vector.tensor_copy(tb2[:], bfu[:])
                nc.sync.dma_start(dbg_hfu[:], tb2[:])
            dbg_done = [False]

            # progress-ordered batches: by starting block, then stream
            border = sorted(
                [(q, b) for q in range(QS) for b in range(nbatch_q[q])],
                key=lambda qb: (int(blockof[qb[0]][min(qb[1] * BCH,
                                len(blockof[qb[0]]) - 1)]), qb[0]))
            for (q, b) in border:
                    prefetch(q, b + PREFETCH)
                    G = gtiles[(q, b)]
                    nch = min(BCH, int(SK_q[q]) - b * BCH)
                    gc0 = int(off_q[q]) + b * BCH         # first chunk col
                    esc_ps = psp.tile([P, 2 * BCH], f32, tag="escp", bufs=2)
                    eqT_bt = sb.tile([P, BCH * P], f8, tag="eqt", bufs=3)
                    nc.sync.dma_start(
                        eqT_bt[:, 0:nch * 128],
                        eqT_in[:, gc0 * 128:(gc0 + nch) * 128])
                    eqN_bt = sb.tile([P, BCH * P], f8, tag="eqn", bufs=3)
                    nc.sync.dma_start(
                        eqN_bt[:, 0:nch * 128],
                        eqN_in[:, gc0 * 128:(gc0 + nch) * 128])
                    for k in range(nch):
                        ci = b * BCH + k                  # stream chunk idx
                        w = int(blockof[q][ci])
                        nc.tensor.matmul(
                            esc_ps[:, 2 * k:2 * k + 2],
                            lhsT=eqT_bt[:, k * 128:(k + 1) * 128],
                            rhs=sc_acc[:, 4 * w + 2:4 * w + 4],
                            start=True, stop=True)
                    esc_sb = sb.tile([P, 2 * BCH], f32, tag="escs", bufs=3)
                    nc.vector.tensor_tensor(
                        out=esc_sb[:, 0:2 * nch],
                        in0=esc_ps[:, 0:2 * nch],
                        in1=asrc_sb[:, gc0 * 2:(gc0 + nch) * 2], op=OP.add)
                    t02 = sb.tile([P, 2 * BCH], f32, tag="t02", bufs=3)
                    nc.vector.tensor_scalar(
                        out=t02[:, 0:2 * nch], in0=esc_sb[:, 0:2 * nch],
                        scalar1=NEG_SLOPE, scalar2=None, op0=OP.mult)
                    lr = sb.tile([P, 2 * BCH], f32, tag="lr", bufs=3)
                    nc.vector.tensor_tensor(
                        out=lr[:, 0:2 * nch], in0=t02[:, 0:2 * nch],
                        in1=esc_sb[:, 0:2 * nch], op=OP.max)
                    expv = sb.tile([P, 2 * BCH], f32, tag="expv", bufs=3)
                    nc.scalar.activation(expv[:, 0:2 * nch],
                                         lr[:, 0:2 * nch], AF.Exp)
                    rhs = sb.tile([P, BCH * 130], f16, tag="rhs", bufs=3)
                    nc.vector.tensor_copy(
                        rhs[:].rearrange(
                            "p (k d) -> p k d", d=130)[:, 0:nch, 128:130],
                        expv[:].rearrange(
                            "p (k d) -> p k d", d=2)[:, 0:nch, :])
                    for k in range(nch):
                        ci = b * BCH + k
                        w = int(blockof[q][ci])
                        nc.vector.tensor_scalar(
                            out=rhs[:, 130 * k:130 * k + C],
                            in0=G[:, ROWW * k:ROWW * k + C],
                            scalar1=expv[:, 2 * k:2 * k + 1], scalar2=None,
                            op0=OP.mult)
                        nc.scalar.activation(
                            rhs[:, 130 * k + C:130 * k + HC],
                            G[:, ROWW * k + C:ROWW * k + HC],
                            AF.Copy, scale=expv[:, 2 * k + 1:2 * k + 2])
                        agg_t, ao = agg_slice(w)
                        first = w not in started
                        started.add(w)
                        nc.tensor.matmul(
                            agg_t[:, ao:ao + HC + 2],
                            lhsT=eqN_bt[:, k * 128:(k + 1) * 128],
                            rhs=rhs[:, 130 * k:130 * (k + 1)],
                            start=first, stop=False)
                        remaining[w] -= 1
                        if remaining[w] == 0:
                            finalize(w)
                    if _dbg and q == 0 and b == 0 and not dbg_done[0]:
                        dbg_done[0] = True
                        tg = sb.tile([P, BCH * ROWW], f32, tag="dbgg", bufs=1)
                        nc.vector.tensor_copy(tg[:], G[:])
                        nc.sync.dma_start(dbg_g[:], tg[:])
                        te = sb.tile([P, 2 * BCH], f32, tag="dbge", bufs=1)
                        nc.vector.tensor_copy(te[:], esc_sb[:])
                        nc.sync.dma_start(dbg_esc[:], te[:])
                        tr = sb.tile([P, BCH * 130], f32, tag="dbgr", bufs=1)
                        nc.vector.tensor_copy(tr[:], rhs[:])
                        nc.sync.dma_start(dbg_rhs[:], tr[:])

            if _dbg:
                for w in range(NB):
                    finp = sb.tile([P, HC], f32, tag="dbgp", bufs=3)
                    nc.vector.tensor_copy(
                        finp[:], out_acc[:, w * HC:(w + 1) * HC])
                    nc.sync.dma_start(
                        dbg_pre[w * 128:(w + 1) * 128, :], finp[:])

            # ---------------- BN epilogue ----------------
            st_sb = sb.tile([1, 2 * HC], f32, tag="st", bufs=1)
            nc.vector.tensor_copy(st_sb[:], stats_ps[:])
            st_loc = dram.tile([1, 2 * HC], f32)
            st_glob = dram.tile([1, 2 * HC], f32)
            nc.sync.dma_start(st_loc[:], st_sb[:])
            nc.gpsimd.collective_compute(
                "AllReduce", OP.add,
                replica_groups=[list(range(NCORES))],
                ins=[st_loc[:].opt()], outs=[st_glob[:].opt()])
            st_g = sb.tile([1, 2 * HC], f32, tag="stg", bufs=1)
            nc.sync.dma_start(st_g[:], st_glob[:])

            sc2 = sb.tile([1, 2 * HC], f32, tag="sc2", bufs=1)
            mrow = sb.tile([1, HC], f32, tag="mrow", bufs=1)
            nc.vector.tensor_scalar(out=mrow[:], in0=st_g[:, 0:HC],
                                    scalar1=1.0 / N, scalar2=None,
                                    op0=OP.mult)
            vrow = sb.tile([1, HC], f32, tag="vrow", bufs=1)
            nc.vector.tensor_scalar(out=vrow[:], in0=st_g[:, HC:2 * HC],
                                    scalar1=1.0 / N, scalar2=None,
                                    op0=OP.mult)
            m2 = sb.tile([1, HC], f32, tag="m2", bufs=1)
            nc.vector.tensor_tensor(out=m2[:], in0=mrow[:], in1=mrow[:],
                                    op=OP.mult)
            nc.vector.tensor_tensor(out=vrow[:], in0=vrow[:], in1=m2[:],
                                    op=OP.subtract)
            nc.vector.tensor_scalar(out=vrow[:], in0=vrow[:],
                                    scalar1=BN_EPS, scalar2=None, op0=OP.add)
            rinv = sb.tile([1, HC], f32, tag="rinv", bufs=1)
            nc.vector.reciprocal(rinv[:], vrow[:])
            rstd = sb.tile([1, HC], f32, tag="rstd", bufs=1)
            nc.scalar.activation(rstd[:], rinv[:], AF.Sqrt)
            nc.vector.tensor_tensor(out=sc2[:, 0:HC], in0=gbb_sb[:, 0:HC],
                                    in1=rstd[:], op=OP.mult)
            msc = sb.tile([1, HC], f32, tag="msc", bufs=1)
            nc.vector.tensor_tensor(out=msc[:], in0=mrow[:],
                                    in1=sc2[:, 0:HC], op=OP.mult)
            nc.vector.tensor_tensor(out=sc2[:, HC:2 * HC],
                                    in0=gbb_sb[:, HC:2 * HC],
                                    in1=msc[:], op=OP.subtract)
            bc_ps = psp.tile([P, 2 * HC], f32, tag="escp", bufs=2)
            nc.tensor.matmul(bc_ps[:], lhsT=ones_row[:], rhs=sc2[:],
                             start=True, stop=True)
            bc_sb = sb.tile([P, 2 * HC], f32, tag="bc", bufs=1)
            nc.vector.tensor_copy(bc_sb[:], bc_ps[:])

            FGRP = 14
            for g in range(NB // FGRP):
                fin = sb.tile([P, FGRP * HC], f32, tag="fin", bufs=2)
                for v in range(FGRP):
                    w = g * FGRP + v
                    nc.vector.tensor_tensor(
                        out=fin[:, v * HC:(v + 1) * HC],
                        in0=out_acc[:, w * HC:(w + 1) * HC],
                        in1=bc_sb[:, 0:HC], op=OP.mult)
                    nc.vector.tensor_tensor(
                        out=fin[:, v * HC:(v + 1) * HC],
                        in0=fin[:, v * HC:(v + 1) * HC],
                        in1=bc_sb[:, HC:2 * HC], op=OP.add)
                nc.sync.dma_start(
                    out_dram[g * FGRP * 128:(g + 1) * FGRP * 128, :]
                    .rearrange("(v j) d -> j v d", j=P),
                    fin[:].rearrange("p (v d) -> p v d", d=HC))

    lower_extended_insts(nc)
    _split_waits(nc, mybir)
    return nc


_CACHE = {}


def kernel(**inputs):
    x = inputs["x"]
    edge_index = inputs["edge_index"]
    W = inputs["W"]
    att_src = inputs["att_src"]
    att_dst = inputs["att_dst"]
    bias = inputs["bias"]
    gamma = inputs["gamma"]
    beta = inputs["beta"]

    per_core, meta = _host_prep(x, edge_index, W, att_src, att_dst,
                                bias, gamma, beta)
    has_bias = bool(np.any(np.asarray(bias) != 0))

    import os as _os
    key = ("prog2", tuple(meta["K"].reshape(-1).tolist()), has_bias,
           bool(_os.environ.get("KERNEL_DEBUG")))
    if key in _CACHE:
        nc = _CACHE[key]
    else:
        nc = _build_program(meta, has_bias)
        _CACHE[key] = nc

    from concourse.bass_utils import run_bass_kernel_spmd
    res = run_bass_kernel_spmd(nc, per_core, core_ids=list(range(NCORES)))

    out = np.zeros((N, HC), dtype=np.float32)
    for c in range(NCORES):
        shard = res.results[c]["out_shard"]          # [NSH, HC] rank-ordered
        order = meta["orders"][c]
        out[c * NSH_RAW + order] = shard[:NSH_RAW]
    return out


# revision 32
# speedup vs baseline: 1.7357x; 1.1112x over previous
"""GAT (2-head, 64-ch) + BatchNorm message passing on 8 Trainium2 cores.

Dst-node graph parallel: 12500 dst nodes/core (98 blocks x 128, in-degree
sorted so per-block edge counts are uniform across cores). Phase 0 computes
h_aug = x @ [W | W@att_src^T | W@att_dst^T] per shard into 512-byte table
rows [h(128f16) | a_src(2) | a_dst(2) | pad]; the table is AllGathered in 4
rank-quarter pieces so quartile-q gathers can start as soon as piece q
lands. Edges are routed to the dst core, bucketed per (dst block, src
quartile) padded to 128-edge chunks (uniform chunk grid across cores), and
gathered 8 chunks (1024 rows) per dma_gather on 4 SWDGE queues with
prefetch. Per chunk: a_dst expand via one PE matmul with a host-uploaded
transposed one-hot (eqT); esc/leaky-relu/exp batched per 8-chunk batch;
w-scaled values built per head on DVE/ACT; one PE matmul per chunk
accumulates numerator and denominator into the block PSUM. Block finalize
adds the self-loop (scores straight from phase-0 columns), normalizes,
applies ReLU, and accumulates BN stats; stats are AllReduced and the BN
affine applied in a final pass.
"""
import sys
sys.path.insert(0, "/opt/trn_rl_repo")
import numpy as np

N = 100_000
F = 128
H = 2
C = 64
HC = H * C
NEG_SLOPE = 0.2
BN_EPS = 1e-5
NCORES = 8
NSH_RAW = 12_500
NSH = 12_544            # 98 * 128
NB = NSH // 128         # 98
QS = 4
# block-aligned shard quarters (ranks); quartile tables are 8x these rows
QSH_P = [3200, 3200, 3072, 3072]
PS_P = [0, 3200, 6400, 9472, 12544]
P = 128
ROWW = 128              # table row width in f16 (256 B)
BCH = 16                # chunks per gather batch (2048 rows)
BROWS = BCH * P
PADVAL = 200.0
PREFETCH = 3


def _host_prep(x, edge_index, W, att_src, att_dst, bias, gamma, beta):
    src = np.asarray(edge_index[0]).astype(np.int64)
    dst = np.asarray(edge_index[1]).astype(np.int64)
    x = np.asarray(x, dtype=np.float32)
    W = np.asarray(W, dtype=np.float32)
    att_src = np.asarray(att_src, dtype=np.float32)
    att_dst = np.asarray(att_dst, dtype=np.float32)

    W_aug = np.zeros((F, 132), dtype=np.float32)
    W_aug[:, 0:HC] = W
    W_aug[:, HC:HC + 2] = np.einsum(
        "fhc,hc->fh", W.reshape(F, H, C), att_src)
    W_aug[:, HC + 2:HC + 4] = np.einsum(
        "fhc,hc->fh", W.reshape(F, H, C), att_dst)
    asrc_all = x @ W_aug[:, HC:HC + 2]                   # [N, 2] f32

    gbb = np.zeros((1, 3 * HC), dtype=np.float32)
    gbb[0, 0:HC] = np.asarray(gamma, dtype=np.float32).reshape(-1)
    gbb[0, HC:2 * HC] = np.asarray(beta, dtype=np.float32).reshape(-1)
    gbb[0, 2 * HC:] = np.asarray(bias, dtype=np.float32).reshape(-1)

    # per-core in-degree rank (degree-sorted blocks)
    orders, ranks = [], []
    for c in range(NCORES):
        m = (dst // NSH_RAW) == c
        d_loc = dst[m] - c * NSH_RAW
        deg = np.bincount(d_loc, minlength=NSH_RAW)
        order = np.argsort(-deg, kind="stable")
        rank = np.empty(NSH_RAW, dtype=np.int64)
        rank[order] = np.arange(NSH_RAW)
        orders.append(order)
        ranks.append(rank)

    ps = np.asarray(PS_P, dtype=np.int64)
    qsh = np.asarray(QSH_P, dtype=np.int64)

    # per-core edge bucketing by (dst block, src quartile)
    core_ed = []
    cnts = np.zeros((NCORES, NB, QS), dtype=np.int64)
    for c in range(NCORES):
        m = (dst // NSH_RAW) == c
        s_c = src[m]
        r_d = ranks[c][dst[m] - c * NSH_RAW]
        w = r_d // 128
        j = r_d % 128
        cs = s_c // NSH_RAW
        r_s_local = np.empty(len(s_c), dtype=np.int64)
        for c2 in range(NCORES):
            mm = cs == c2
            r_s_local[mm] = ranks[c2][s_c[mm] - c2 * NSH_RAW]
        q = (np.searchsorted(ps, r_s_local, side="right") - 1).astype(np.int64)
        idx16 = cs * qsh[q] + (r_s_local - ps[q])
        core_ed.append((w, j, q, idx16, s_c))
        np.add.at(cnts[c], (w, q), 1)

    K = ((cnts.max(axis=0) + 127) // 128).astype(np.int64)   # [NB, QS]
    SK_q = K.sum(axis=0)                                     # chunks/stream
    TOTCH = int(SK_q.sum())
    nbatch_q = [int((SK_q[q] + BCH - 1) // BCH) for q in range(QS)]
    base_wq = np.zeros((NB, QS), dtype=np.int64)             # chunk base of (w,q)
    for q in range(QS):
        base_wq[1:, q] = np.cumsum(K[:-1, q])
    off_q = np.zeros(QS, dtype=np.int64)                     # stream col offset
    off_q[1:] = np.cumsum(SK_q[:-1])

    per_core = []
    core_src = []
    for c in range(NCORES):
        core_src.append(None)
    for c in range(NCORES):
        w, j, q, idx16, s_c = core_ed[c]
        ordv = np.lexsort((idx16, j, w + NB * q))
        wq, jq, qq, iq = w[ordv], j[ordv], q[ordv], idx16[ordv]
        core_src[c] = s_c[ordv]

        import ml_dtypes
        f8 = ml_dtypes.float8_e4m3
        idx_streams = [np.zeros(nbatch_q[s] * BROWS, dtype=np.int16)
                       for s in range(QS)]
        eqT = np.zeros((128, TOTCH * 128), dtype=f8)
        eqN = np.zeros((128, TOTCH * 128), dtype=f8)
        asrcS = np.zeros((TOTCH * 128, 2), dtype=np.float16)
        src_glob = src  # global src ids (closure)
        sg = core_src[c]
        for s in range(QS):
            ms = qq == s
            ws, js, is_, gsrc = wq[ms], jq[ms], iq[ms], sg[ms]
            blo = np.searchsorted(ws, np.arange(NB))
            bhi = np.searchsorted(ws, np.arange(NB) + 1)
            for wv in range(NB):
                a, b = int(blo[wv]), int(bhi[wv])
                ne = b - a
                if ne == 0:
                    continue
                sbase = int(base_wq[wv, s]) * 128          # row in stream s
                idx_streams[s][sbase:sbase + ne] = is_[a:b].astype(np.int16)
                gch = (int(off_q[s]) + int(base_wq[wv, s])) * 128
                eqT[js[a:b], gch + np.arange(ne)] = 1.0
                # eq rows: partition = edge slot within its chunk
                esl = (gch + np.arange(ne)) % 128
                ech = (gch + np.arange(ne)) // 128
                eqN[esl, ech * 128 + js[a:b]] = 1.0
                asrcS[gch:gch + ne] = asrc_all[gsrc[a:b]].astype(np.float16)

        # [128 slot, TOTCH*2] layout: col ch*2+h
        asrc_t = np.ascontiguousarray(
            asrcS.reshape(TOTCH, 128, 2).transpose(1, 0, 2).reshape(
                128, TOTCH * 2))
        inp = {
            "xT": None,          # filled below
            "W_aug": W_aug,
            "gbb": gbb,
            "asrcS": asrc_t,
            "eqT": eqT,
            "eqN": eqN,
        }
        for s in range(QS):
            lin = idx_streams[s]
            wrapped = lin.reshape(-1, 16).T                  # [16, nb*64]
            arr = np.zeros((P, nbatch_q[s] * (BROWS // 16)), dtype=np.int16)
            for grp in range(8):
                arr[grp * 16:(grp + 1) * 16, :] = wrapped
            inp[f"idx16_{s}"] = arr

        xs = np.zeros((NSH, F), dtype=np.float32)
        xs[:NSH_RAW] = x[c * NSH_RAW:(c + 1) * NSH_RAW][orders[c]]
        inp["xT"] = np.ascontiguousarray(xs.T).astype(np.float16)
        per_core.append(inp)

    meta = dict(K=K, SK_q=SK_q, TOTCH=TOTCH, nbatch_q=nbatch_q,
                base_wq=base_wq, off_q=off_q, orders=orders)
    return per_core, meta


def _split_waits(nc, mybir, keep=1):
    """Walrus accepts at most one sem-wait on DMA/CTRL pseudo instructions;
    hoist excess waits onto InstEventSemaphore."""
    for f in nc.m.functions:
        for bb in f.blocks:
            new = []
            for ins in bb.instructions:
                si = ins.sync_info
                if si is not None and si.on_wait and len(si.on_wait) > keep:
                    for jj, wcond in enumerate(list(si.on_wait)[:-keep]):
                        w = mybir.InstEventSemaphore(
                            name=f"{ins.name}-ws{jj}", ins=[], outs=[])
                        w.engine = ins.engine
                        w.sync_info = mybir.SyncInfo(
                            on_wait=[wcond], on_update=[])
                        new.append(w)
                    ins.sync_info = mybir.SyncInfo(
                        on_wait=list(si.on_wait)[-keep:],
                        on_update=list(si.on_update))
                new.append(ins)
            bb.instructions[:] = new


def _build_program(meta, has_bias):
    import concourse.bass as bass
    import concourse.mybir as mybir
    import concourse.tile as tile
    from concourse.masks import make_identity
    from concourse.library_config import mlp as mlp_lib
    from concourse.library_overlay import lower_extended_insts

    K = meta["K"]; SK_q = meta["SK_q"]; TOTCH = meta["TOTCH"]
    nbatch_q = meta["nbatch_q"]; base_wq = meta["base_wq"]
    off_q = meta["off_q"]
    f16 = mybir.dt.float16
    f32 = mybir.dt.float32
    f8 = mybir.dt.float8e4
    i16 = mybir.dt.int16
    AF = mybir.ActivationFunctionType
    OP = mybir.AluOpType

    nc = bass.Bass(num_devices=NCORES, num_swdge_queues=QS)
    xT_in = nc.dram_tensor("xT", [F, NSH], f16, kind="ExternalInput")
    W_in = nc.dram_tensor("W_aug", [F, 132], f32, kind="ExternalInput")
    gbb_in = nc.dram_tensor("gbb", [1, 3 * HC], f32, kind="ExternalInput")
    asrc_in = nc.dram_tensor("asrcS", [P, TOTCH * 2], f16,
                             kind="ExternalInput")
    eqT_in = nc.dram_tensor("eqT", [P, TOTCH * 128], f8,
                            kind="ExternalInput")
    eqN_in = nc.dram_tensor("eqN", [P, TOTCH * 128], f8,
                            kind="ExternalInput")
    idx_in = [nc.dram_tensor(f"idx16_{q}", [P, nbatch_q[q] * (BROWS // 16)],
                             i16, kind="ExternalInput") for q in range(QS)]
    out_dram = nc.dram_tensor("out_shard", [NSH, HC], f32,
                              kind="ExternalOutput")
    import os as _os
    _dbg = bool(_os.environ.get("KERNEL_DEBUG"))
    if _dbg:
        dbg_g = nc.dram_tensor("dbg_g", [P, BCH * ROWW], f32,
                               kind="ExternalOutput")
        dbg_esc = nc.dram_tensor("dbg_esc", [P, 2 * BCH], f32,
                                 kind="ExternalOutput")
        dbg_rhs = nc.dram_tensor("dbg_rhs", [P, BCH * 130], f32,
                                 kind="ExternalOutput")
        dbg_pre = nc.dram_tensor("dbg_pre", [NSH, HC], f32,
                                 kind="ExternalOutput")
        dbg_hsh = nc.dram_tensor("dbg_hsh", [P, ROWW], f32,
                                 kind="ExternalOutput")
        dbg_hfu = nc.dram_tensor("dbg_hfu", [P, ROWW], f32,
                                 kind="ExternalOutput")

    # chunk -> block map per stream
    blockof = [np.repeat(np.arange(NB), K[:, q]) for q in range(QS)]
    # first/last chunk of each block (global over the 4 streams' chunklists)
    remaining0 = K.sum(axis=1)

    with tile.TileContext(nc) as tc:
        with tc.tile_pool(name="cst", bufs=1) as cst, \
             tc.tile_pool(name="sb", bufs=2) as sb, \
             tc.tile_pool(name="ps", bufs=1, space="PSUM") as psp, \
             tc.tile_pool(name="dram", bufs=1, space="DRAM") as dram:

            ident = cst.tile([P, P], f16)
            make_identity(nc, ident[:])
            iota_i = cst.tile([P, P], mybir.dt.int32)
            nc.gpsimd.iota(iota_i[:], pattern=[[1, P]], channel_multiplier=0)
            iota16 = cst.tile([P, P], f16)
            nc.vector.tensor_copy(iota16[:], iota_i[:])
            ones16 = cst.tile([P, 1], f16)
            nc.vector.memset(ones16[:], 1.0)
            ones_row = cst.tile([1, P], f32)
            nc.vector.memset(ones_row[:], 1.0)
            W_f32 = cst.tile([F, 132], f32)
            nc.sync.dma_start(W_f32[:], W_in[:])
            W_sb = cst.tile([F, 132], f16)
            nc.vector.tensor_copy(W_sb[:], W_f32[:])
            gbb_sb = cst.tile([1, 3 * HC], f32)
            nc.sync.dma_start(gbb_sb[:], gbb_in[:])
            asrc_sb = cst.tile([P, TOTCH * 2], f16)
            nc.sync.dma_start(asrc_sb[:], asrc_in[:])
            idx_sb = []
            for q in range(QS):
                t = cst.tile([P, nbatch_q[q] * (BROWS // 16)], i16,
                             name=f"idxsb{q}")
                nc.sync.dma_start(t[:], idx_in[q][:])
                idx_sb.append(t)
            sc_acc = cst.tile([P, NB * 4], f16)
            out_acc = cst.tile([P, NB * HC], f16)

            nc.gpsimd.load_library(mlp_lib)

            # ---------------- phase 0: augmented h table ----------------
            h_shard_p = [dram.tile([QSH_P[p], ROWW], f16, name=f"hsh{p}")
                         for p in range(QS)]
            h_full_p = [dram.tile([NCORES * QSH_P[p], ROWW], f16,
                                  name=f"hfu{p}") for p in range(QS)]
            ag_after = {(PS_P[p + 1] - 1) // 128: p for p in range(QS)}

            nidx_regs = {}

            def reg_for(v):
                if v not in nidx_regs:
                    nidx_regs[v] = nc.gpsimd.to_reg(v)
                return nidx_regs[v]

            gtiles = {}

            def issue_gather(q, b):
                gt = sb.tile([P, BCH * ROWW], f16, tag=f"g{q}", bufs=4,
                             name=f"g{q}_{b}")
                nc.gpsimd.dma_gather(
                    out_ap=gt[:].rearrange("p (k d) -> p k d", d=ROWW),
                    in_ap=h_full_p[q][:, :],
                    idxs_ap=idx_sb[q][:, b * (BROWS // 16):
                                      (b + 1) * (BROWS // 16)],
                    num_idxs=BROWS,
                    num_idxs_reg=reg_for(BROWS),
                    elem_size=ROWW,
                    single_packet=False,
                    queue_num=q)
                gtiles[(q, b)] = gt

            next_issue = [0] * QS

            def prefetch(q, upto):
                while next_issue[q] <= min(upto, nbatch_q[q] - 1):
                    issue_gather(q, next_issue[q])
                    next_issue[q] += 1

            GRP = 14                       # blocks per phase-0 group
            for g in range(NB // GRP):
                xt = sb.tile([P, GRP * P], f16, tag="xt", bufs=2)
                nc.sync.dma_start(
                    xt[:], xT_in[:, g * GRP * 128:(g + 1) * GRP * 128])
                h_sb = sb.tile([P, GRP * ROWW], f16, tag="hsb", bufs=2)
                for v in range(GRP):
                    w = g * GRP + v
                    h_ps = psp.tile([P, 132], f32, tag="escp", bufs=2)
                    nc.tensor.matmul(h_ps[:], lhsT=xt[:, v * 128:
                                                      (v + 1) * 128],
                                     rhs=W_sb[:], start=True, stop=True)
                    nc.scalar.copy(
                        h_sb[:, v * ROWW:v * ROWW + HC], h_ps[:, 0:HC])
                    nc.vector.tensor_copy(sc_acc[:, 4 * w:4 * w + 4],
                                          h_ps[:, 128:132])
                # write group rows [1792g, 1792(g+1)) split by quarter piece
                r0, r1 = g * GRP * 128, (g + 1) * GRP * 128
                hv = h_sb[:].rearrange("p (v d) -> p v d", d=ROWW)
                for pc in range(QS):
                    a = max(r0, PS_P[pc])
                    bnd = min(r1, PS_P[pc + 1])
                    if a >= bnd:
                        continue
                    nc.sync.dma_start(
                        h_shard_p[pc][a - PS_P[pc]:bnd - PS_P[pc], :]
                        .rearrange("(v j) d -> j v d", j=P),
                        hv[:, (a - r0) // 128:(bnd - r0) // 128, :])
                for w in range(g * GRP, (g + 1) * GRP):
                    if w in ag_after:
                        p = ag_after[w]
                        nc.gpsimd.collective_compute(
                            "AllGather", OP.bypass,
                            replica_groups=[list(range(NCORES))],
                            ins=[h_shard_p[p][:].opt()],
                            outs=[h_full_p[p][:].opt()])
            # rotate across queues so all 4 SWDGE queues get transfers in
            # flight — queue-major issue serializes the rings behind each
            # other's completion waits
            for d in range(PREFETCH + 1):
                for q in range(QS):
                    prefetch(q, d)

            # self-loop scores for all blocks at once
            sc_v = sc_acc[:].rearrange("p (w d) -> p w d", d=4)
            esc_s = cst.tile([P, NB * 2], f32)
            nc.vector.tensor_tensor(
                out=esc_s[:].rearrange("p (w d) -> p w d", d=2),
                in0=sc_v[:, :, 0:2], in1=sc_v[:, :, 2:4], op=OP.add)
            t02_s = cst.tile([P, NB * 2], f32)
            nc.vector.tensor_scalar(out=t02_s[:], in0=esc_s[:],
                                    scalar1=NEG_SLOPE, scalar2=None,
                                    op0=OP.mult)
            nc.vector.tensor_tensor(out=esc_s[:], in0=t02_s[:],
                                    in1=esc_s[:], op=OP.max)
            expv_s = cst.tile([P, NB * 2], f32)
            nc.scalar.activation(expv_s[:], esc_s[:], AF.Exp)

            if has_bias:
                bias_ps = psp.tile([P, HC], f32, tag="escp", bufs=2)
                nc.tensor.matmul(bias_ps[:], lhsT=ones_row[:],
                                 rhs=gbb_sb[:, 2 * HC:3 * HC],
                                 start=True, stop=True)
                bias_bc = cst.tile([P, HC], f32)
                nc.vector.tensor_copy(bias_bc[:], bias_ps[:])

            stats_ps = psp.tile([1, 2 * HC], f32, tag="stats", bufs=1)
            nc.vector.memset(stats_ps[:], 0.0)

            # ---------------- main loop (batch-major) ----------------
            agg_tiles = {}           # triple t -> psum tile [P, 3*130]
            triple_left = {}
            started = set()
            remaining = remaining0.copy()
            nfin = [0]

            def agg_slice(w):
                if w not in agg_tiles:
                    agg_tiles[w] = psp.tile([P, HC + 2], f32,
                                            tag="agg", bufs=5,
                                            name=f"agg{w}")
                return agg_tiles[w], 0

            def finalize(w):
                gself = sb.tile([P, HC], f16, tag="gself", bufs=3)
                p0 = next(p for p in range(QS)
                          if PS_P[p] <= w * 128 < PS_P[p + 1])
                r0 = w * 128 - PS_P[p0]
                nc.sync.dma_start(gself[:], h_shard_p[p0][r0:r0 + 128, :])
                rhs_s = sb.tile([P, 130], f16, tag="rhss", bufs=2)
                nc.vector.tensor_scalar(
                    out=rhs_s[:, 0:C], in0=gself[:, 0:C],
                    scalar1=expv_s[:, 2 * w:2 * w + 1], scalar2=None,
                    op0=OP.mult)
                nc.scalar.activation(
                    rhs_s[:, C:HC], gself[:, C:HC], AF.Copy,
                    scale=expv_s[:, 2 * w + 1:2 * w + 2])
                nc.vector.tensor_copy(rhs_s[:, HC:HC + 2],
                                      expv_s[:, 2 * w:2 * w + 2])
                agg_t, ao = agg_slice(w)
                nc.tensor.matmul(agg_t[:, ao:ao + HC + 2], lhsT=ident[:],
                                 rhs=rhs_s[:], start=False, stop=True)
                recip = sb.tile([P, 2], f32, tag="recip", bufs=3)
                nc.vector.reciprocal(recip[:], agg_t[:, ao + HC:ao + HC + 2])
                oslice = out_acc[:, w * HC:(w + 1) * HC]
                for h in range(H):
                    if has_bias:
                        tmp = sb.tile([P, C], f32, tag="tmpb", bufs=2)
                        nc.vector.tensor_scalar(
                            out=tmp[:],
                            in0=agg_t[:, ao + C * h:ao + C * (h + 1)],
                            scalar1=recip[:, h:h + 1], scalar2=None,
                            op0=OP.mult)
                        nc.vector.tensor_tensor(
                            out=tmp[:], in0=tmp[:],
                            in1=bias_bc[:, C * h:C * (h + 1)], op=OP.add)
                        nc.vector.tensor_scalar(
                            out=oslice[:, C * h:C * (h + 1)], in0=tmp[:],
                            scalar1=0.0, scalar2=None, op0=OP.max)
                    else:
                        nc.vector.tensor_scalar(
                            out=oslice[:, C * h:C * (h + 1)],
                            in0=agg_t[:, ao + C * h:ao + C * (h + 1)],
                            scalar1=recip[:, h:h + 1], scalar2=0.0,
                            op0=OP.mult, op1=OP.max)
                agg_tiles.pop(w)
                sq_t = sb.tile([P, HC], f16, tag="sq", bufs=2)
                nc.vector.tensor_tensor(out=sq_t[:], in0=oslice, in1=oslice,
                                        op=OP.mult)
                nc.tensor.matmul(stats_ps[:, 0:HC], lhsT=ones16[:],
                                 rhs=oslice, start=False,
                                 stop=(nfin[0] == NB - 1))
                nc.tensor.matmul(stats_ps[:, HC:2 * HC], lhsT=ones16[:],
                                 rhs=sq_t[:], start=False,
                                 stop=(nfin[0] == NB - 1))
                nfin[0] += 1

            if _dbg:
                tb = sb.tile([P, ROWW], f32, tag="dbg", bufs=1)
                bsh = sb.tile([P, ROWW], f16, tag="dbgh", bufs=1)
                nc.sync.dma_start(bsh[:], h_shard_p[0][0:128, :])
                nc.vector.tensor_copy(tb[:], bsh[:])
                nc.sync.dma_start(dbg_hsh[:], tb[:])
                tb2 = sb.tile([P, ROWW], f32, tag="dbg2", bufs=1)
                bfu = sb.tile([P, ROWW], f16, tag="dbgh2", bufs=1)
                nc.sync.dma_start(bfu[:], h_full_p[0][QSH_P[0] * 1:
                                                      QSH_P[0] * 1 + 128, :])
                nc.vector.tensor_copy(tb2[:], bfu[:])
                nc.sync.dma_start(dbg_hfu[:], tb2[:])
            dbg_done = [False]

            # progress-ordered batches: by starting block, then stream
            border = sorted(
                [(q, b) for q in range(QS) for b in range(nbatch_q[q])],
                key=lambda qb: (int(blockof[qb[0]][min(qb[1] * BCH,
                                len(blockof[qb[0]]) - 1)]), qb[0]))
            for (q, b) in border:
                    prefetch(q, b + PREFETCH)
                    G = gtiles[(q, b)]
                    nch = min(BCH, int(SK_q[q]) - b * BCH)
                    gc0 = int(off_q[q]) + b * BCH         # first chunk col
                    esc_ps = psp.tile([P, 2 * BCH], f32, tag="escp", bufs=2)
                    eqT_bt = sb.tile([P, BCH * P], f8, tag="eqt", bufs=3)
                    nc.sync.dma_start(
                        eqT_bt[:, 0:nch * 128],
                        eqT_in[:, gc0 * 128:(gc0 + nch) * 128])
                    eqN_bt = sb.tile([P, BCH * P], f8, tag="eqn", bufs=3)
                    nc.sync.dma_start(
                        eqN_bt[:, 0:nch * 128],
                        eqN_in[:, gc0 * 128:(gc0 + nch) * 128])
                    for k in range(nch):
                        ci = b * BCH + k                  # stream chunk idx
                        w = int(blockof[q][ci])
                        nc.tensor.matmul(
                            esc_ps[:, 2 * k:2 * k + 2],
                            lhsT=eqT_bt[:, k * 128:(k + 1) * 128],
                            rhs=sc_acc[:, 4 * w + 2:4 * w + 4],
                            start=True, stop=True)
                    esc_sb = sb.tile([P, 2 * BCH], f32, tag="escs", bufs=3)
                    nc.vector.tensor_tensor(
                        out=esc_sb[:, 0:2 * nch],
                        in0=esc_ps[:, 0:2 * nch],
                        in1=asrc_sb[:, gc0 * 2:(gc0 + nch) * 2], op=OP.add)
                    t02 = sb.tile([P, 2 * BCH], f32, tag="t02", bufs=3)
                    nc.vector.tensor_scalar(
                        out=t02[:, 0:2 * nch], in0=esc_sb[:, 0:2 * nch],
                        scalar1=NEG_SLOPE, scalar2=None, op0=OP.mult)
                    lr = sb.tile([P, 2 * BCH], f32, tag="lr", bufs=3)
                    nc.vector.tensor_tensor(
                        out=lr[:, 0:2 * nch], in0=t02[:, 0:2 * nch],
                        in1=esc_sb[:, 0:2 * nch], op=OP.max)
                    expv = sb.tile([P, 2 * BCH], f32, tag="expv", bufs=3)
                    nc.scalar.activation(expv[:, 0:2 * nch],
                                         lr[:, 0:2 * nch], AF.Exp)
                    rhs = sb.tile([P, BCH * 130], f16, tag="rhs", bufs=3)
                    nc.vector.tensor_copy(
                        rhs[:].rearrange(
                            "p (k d) -> p k d", d=130)[:, 0:nch, 128:130],
                        expv[:].rearrange(
                            "p (k d) -> p k d", d=2)[:, 0:nch, :])
                    for k in range(nch):
                        ci = b * BCH + k
                        w = int(blockof[q][ci])
                        nc.vector.tensor_scalar(
                            out=rhs[:, 130 * k:130 * k + C],
                            in0=G[:, ROWW * k:ROWW * k + C],
                            scalar1=expv[:, 2 * k:2 * k + 1], scalar2=None,
                            op0=OP.mult)
                        nc.scalar.activation(
                            rhs[:, 130 * k + C:130 * k + HC],
                            G[:, ROWW * k + C:ROWW * k + HC],
                            AF.Copy, scale=expv[:, 2 * k + 1:2 * k + 2])
                        agg_t, ao = agg_slice(w)
                        first = w not in started
                        started.add(w)
                        nc.tensor.matmul(
                            agg_t[:, ao:ao + HC + 2],
                            lhsT=eqN_bt[:, k * 128:(k + 1) * 128],
                            rhs=rhs[:, 130 * k:130 * (k + 1)],
                            start=first, stop=False)
                        remaining[w] -= 1
                        if remaining[w] == 0:
                            finalize(w)
                    if _dbg and q == 0 and b == 0 and not dbg_done[0]:
                        dbg_done[0] = True
                        tg = sb.tile([P, BCH * ROWW], f32, tag="dbgg", bufs=1)
                        nc.vector.tensor_copy(tg[:], G[:])
                        nc.sync.dma_start(dbg_g[:], tg[:])
                        te = sb.tile([P, 2 * BCH], f32, tag="dbge", bufs=1)
                        nc.vector.tensor_copy(te[:], esc_sb[:])
                        nc.sync.dma_start(dbg_esc[:], te[:])
                        tr = sb.tile([P, BCH * 130], f32, tag="dbgr", bufs=1)
                        nc.vector.tensor_copy(tr[:], rhs[:])
                        nc.sync.dma_start(dbg_rhs[:], tr[:])

            if _dbg:
                for w in range(NB):
                    finp = sb.tile([P, HC], f32, tag="dbgp", bufs=3)
                    nc.vector.tensor_copy(
                        finp[:], out_acc[:, w * HC:(w + 1) * HC])
                    nc.sync.dma_start(
                        dbg_pre[w * 128:(w + 1) * 128, :], finp[:])

            # ---------------- BN epilogue ----------------
            st_sb = sb.tile([1, 2 * HC], f32, tag="st", bufs=1)
            nc.vector.tensor_copy(st_sb[:], stats_ps[:])
            st_loc = dram.tile([1, 2 * HC], f32)
            st_glob = dram.tile([1, 2 * HC], f32)
            nc.sync.dma_start(st_loc[:], st_sb[:])
            nc.gpsimd.collective_compute(
                "AllReduce", OP.add,
                replica_groups=[list(range(NCORES))],
                ins=[st_loc[:].opt()], outs=[st_glob[:].opt()])
            st_g = sb.tile([1, 2 * HC], f32, tag="stg", bufs=1)
            nc.sync.dma_start(st_g[:], st_glob[:])

            sc2 = sb.tile([1, 2 * HC], f32, tag="sc2", bufs=1)
            mrow = sb.tile([1, HC], f32, tag="mrow", bufs=1)
            nc.vector.tensor_scalar(out=mrow[:], in0=st_g[:, 0:HC],
                                    scalar1=1.0 / N, scalar2=None,
                                    op0=OP.mult)
            vrow = sb.tile([1, HC], f32, tag="vrow", bufs=1)
            nc.vector.tensor_scalar(out=vrow[:], in0=st_g[:, HC:2 * HC],
                                    scalar1=1.0 / N, scalar2=None,
                                    op0=OP.mult)
            m2 = sb.tile([1, HC], f32, tag="m2", bufs=1)
            nc.vector.tensor_tensor(out=m2[:], in0=mrow[:], in1=mrow[:],
                                    op=OP.mult)
            nc.vector.tensor_tensor(out=vrow[:], in0=vrow[:], in1=m2[:],
                                    op=OP.subtract)
            nc.vector.tensor_scalar(out=vrow[:], in0=vrow[:],
                                    scalar1=BN_EPS, scalar2=None, op0=OP.add)
            rinv = sb.tile([1, HC], f32, tag="rinv", bufs=1)
            nc.vector.reciprocal(rinv[:], vrow[:])
            rstd = sb.tile([1, HC], f32, tag="rstd", bufs=1)
            nc.scalar.activation(rstd[:], rinv[:], AF.Sqrt)
            nc.vector.tensor_tensor(out=sc2[:, 0:HC], in0=gbb_sb[:, 0:HC],
                                    in1=rstd[:], op=OP.mult)
            msc = sb.tile([1, HC], f32, tag="msc", bufs=1)
            nc.vector.tensor_tensor(out=msc[:], in0=mrow[:],
                                    in1=sc2[:, 0:HC], op=OP.mult)
            nc.vector.tensor_tensor(out=sc2[:, HC:2 * HC],
                                    in0=gbb_sb[:, HC:2 * HC],
                                    in1=msc[:], op=OP.subtract)
            bc_ps = psp.tile([P, 2 * HC], f32, tag="escp", bufs=2)
            nc.tensor.matmul(bc_ps[:], lhsT=ones_row[:], rhs=sc2[:],
                             start=True, stop=True)
            bc_sb = sb.tile([P, 2 * HC], f32, tag="bc", bufs=1)
            nc.vector.tensor_copy(bc_sb[:], bc_ps[:])

            FGRP = 14
            for g in range(NB // FGRP):
                fin = sb.tile([P, FGRP * HC], f32, tag="fin", bufs=2)
                for v in range(FGRP):
                    w = g * FGRP + v
                    nc.vector.tensor_tensor(
                        out=fin[:, v * HC:(v + 1) * HC],
                        in0=out_acc[:, w * HC:(w + 1) * HC],
                        in1=bc_sb[:, 0:HC], op=OP.mult)
                    nc.vector.tensor_tensor(
                        out=fin[:, v * HC:(v + 1) * HC],
                        in0=fin[:, v * HC:(v + 1) * HC],
                        in1=bc_sb[:, HC:2 * HC], op=OP.add)
                nc.sync.dma_start(
                    out_dram[g * FGRP * 128:(g + 1) * FGRP * 128, :]
                    .rearrange("(v j) d -> j v d", j=P),
                    fin[:].rearrange("p (v d) -> p v d", d=HC))

    lower_extended_insts(nc)
    _split_waits(nc, mybir)
    return nc


_CACHE = {}


def kernel(**inputs):
    x = inputs["x"]
    edge_index = inputs["edge_index"]
    W = inputs["W"]
    att_src = inputs["att_src"]
    att_dst = inputs["att_dst"]
    bias = inputs["bias"]
    gamma = inputs["gamma"]
    beta = inputs["beta"]

    per_core, meta = _host_prep(x, edge_index, W, att_src, att_dst,
                                bias, gamma, beta)
    has_bias = bool(np.any(np.asarray(bias) != 0))

    import os as _os
    key = ("prog2", tuple(meta["K"].reshape(-1).tolist()), has_bias,
           bool(_os.environ.get("KERNEL_DEBUG")))
    if key in _CACHE:
        nc = _CACHE[key]
    else:
        nc = _build_program(meta, has_bias)
        _CACHE[key] = nc

    from concourse.bass_utils import run_bass_kernel_spmd
    res = run_bass_kernel_spmd(nc, per_core, core_ids=list(range(NCORES)))

    out = np.zeros((N, HC), dtype=np.float32)
    for c in range(NCORES):
        shard = res.results[c]["out_shard"]          # [NSH, HC] rank-ordered
        order = meta["orders"][c]
        out[c * NSH_RAW + order] = shard[:NSH_RAW]
    return out
